# revision 5
# baseline (speedup 1.0000x reference)
"""AttentionContext kernel for Trainium2, data-parallel over batch on 8 cores.

Reference computation (B=64, T=2048, D=512 everywhere):
    phi_s = s @ phi_w.T + phi_b                  # [B, D]
    psi_h = einsum('bth,ah->bta', h, psi_w) + psi_b
    e     = einsum('ba,bta->bt', phi_s, psi_h)   # [B, T]
    alpha = softmax(e, axis=-1)
    c     = alpha * h.sum(-1)                    # [B, T]

Algebraic restructuring:
    e[b,t] = (phi_s[b] @ psi_w) . h[b,t] + const(b); softmax drops const(b).
    w = s @ (phi_w.T @ psi_w) + phi_b @ psi_w    # [B, D], tiny, on PE

Streaming stage: h is cast fp32->fp16 inside the SWDGE DMA (inline cast in
the DMA engines, HBM read traffic unchanged, SBUF writes halved). With
16-bit operands the DVE runs scalar_tensor_tensor in 2x_1P packed mode
(~2x the fp32 rate), so e[b,t] = h16 . w16 and half of the hsum tiles fit
on the DVE; the other hsum half runs on ScalarE activation-accumulate.
fp16 (not bf16) keeps the softmax-amplified e error ~1.5e-3 Frobenius.

w never round-trips DRAM: it is broadcast down partitions with a K=1 PE
matmul per batch and cast to fp16 by ScalarE during PSUM evacuation.
"""

import numpy as np

import concourse.bass as bass
import concourse.bacc as bacc
import concourse.tile as tile
from concourse import mybir
from concourse import bass_utils
from concourse.masks import make_identity

FP = mybir.dt.float32
F16 = mybir.dt.float16
ALU = mybir.AluOpType
AF = mybir.ActivationFunctionType

N_CORES = 8
B_LOC = 8          # batches per core
T = 2048
D = 512
P = 128
KC = D // P        # 4 contraction chunks of 128
TI = T // P        # 16 t-tiles per batch
SUP = 8            # t-tiles per DMA super-tile
NSUP = TI // SUP   # 2 super-tiles per batch


def _emit(nc, tc):
    s = nc.dram_tensor("s", [B_LOC, D], FP, kind="ExternalInput").ap()
    h = nc.dram_tensor("h", [B_LOC, T, D], FP, kind="ExternalInput").ap()
    phi_w = nc.dram_tensor("phi_w", [D, D], FP, kind="ExternalInput").ap()
    phi_b = nc.dram_tensor("phi_b", [D], FP, kind="ExternalInput").ap()
    psi_w = nc.dram_tensor("psi_w", [D, D], FP, kind="ExternalInput").ap()
    c_out = nc.dram_tensor("c", [B_LOC, T], FP, kind="ExternalOutput").ap()

    with tc.tile_pool(name="consts", bufs=1) as consts:
        # ---------------- stage 0: combined weights ----------------
        phi_w_sb = consts.tile([P, KC, D], FP)   # [a % 128, a // 128, k]
        psi_w_sb = consts.tile([P, KC, D], FP)   # [a % 128, a // 128, m]
        # per-chunk loads: the ac=0 matmul of M_c only needs chunk 0 of each
        # weight, so the PE chain starts ~4x earlier than with 1MB transfers
        for ac in range(KC):
            nc.sync.dma_start(
                out=phi_w_sb[:, ac, :], in_=phi_w[ac * P : (ac + 1) * P, :]
            )
            nc.sync.dma_start(
                out=psi_w_sb[:, ac, :], in_=psi_w[ac * P : (ac + 1) * P, :]
            )
        phi_b_sb = consts.tile([P, KC], FP)      # [a % 128, a // 128]
        nc.sync.dma_start(out=phi_b_sb, in_=phi_b.rearrange("(ac p) -> p ac", p=P))
        s_sb = consts.tile([B_LOC, D], FP)
        nc.sync.dma_start(out=s_sb, in_=s)

        ident = consts.tile([P, P], FP)
        make_identity(nc, ident)
        ones_1x128 = consts.tile([1, P], FP)
        nc.vector.memset(ones_1x128, 1.0)
        neg_1x128 = consts.tile([1, P], FP)
        nc.vector.memset(neg_1x128, -1.0)
        ones_128x1 = consts.tile([P, 1], FP)
        nc.vector.memset(ones_128x1, 1.0)
        ones16 = consts.tile([P, D], F16)
        nc.vector.memset(ones16, 1.0)

        # Warm the ACT exp table set early so the ~2.7us load overlaps.
        tiny = consts.tile([1, 1], FP)
        nc.vector.memset(tiny, 0.0)
        nc.scalar.activation(out=tiny, in_=tiny, func=AF.Exp)

        mc_sb = consts.tile([P, KC, D], FP)      # M_c[k, m], k = kc*128 + p
        v_sb = consts.tile([1, D], FP)           # v[m] = phi_b @ psi_w
        sT_sb = consts.tile([P, KC, B_LOC], FP)  # s.T[k, b]
        w_sb16 = consts.tile([B_LOC, D], F16)    # w[b, m] fp16
        w_rows16 = consts.tile([1, B_LOC, D], F16)  # each w row at partition 0
        w_bc16 = consts.tile([P, B_LOC, D], F16)  # w[b] broadcast down parts
        e_all = consts.tile([P, P], FP)          # e[t%128, b*16 + ti]
        hs_all = consts.tile([P, P], FP)         # hsum, same layout
        exp_all = consts.tile([P, P], FP)        # exp(e - max_b), same layout

        with tc.tile_pool(name="psum0", bufs=2, space="PSUM") as psum0:
            # s.T chunks via PE transpose (emitted first: s arrives fast, so
            # the PE queue has work while the weight chunks stream in)
            for kc in range(KC):
                st_ps = psum0.tile([P, B_LOC], FP, tag="st_ps")
                nc.tensor.transpose(
                    st_ps,
                    in_=s_sb[:, kc * P : (kc + 1) * P],
                    identity=ident[:B_LOC, :B_LOC],
                )
                nc.vector.tensor_copy(out=sT_sb[:, kc, :], in_=st_ps)

            # M_c[k, m] = sum_a phi_w[a, k] * psi_w[a, m]
            for kc in range(KC):
                mc_ps = psum0.tile([P, D], FP, tag="mc_ps")
                for ac in range(KC):
                    nc.tensor.matmul(
                        mc_ps,
                        lhsT=phi_w_sb[:, ac, kc * P : (kc + 1) * P],
                        rhs=psi_w_sb[:, ac, :],
                        start=(ac == 0),
                        stop=(ac == KC - 1),
                    )
                nc.vector.tensor_copy(out=mc_sb[:, kc, :], in_=mc_ps)

            # v[m] = sum_a phi_b[a] * psi_w[a, m]
            v_ps = psum0.tile([1, D], FP, tag="v_ps")
            for ac in range(KC):
                nc.tensor.matmul(
                    v_ps,
                    lhsT=phi_b_sb[:, ac : ac + 1],
                    rhs=psi_w_sb[:, ac, :],
                    start=(ac == 0),
                    stop=(ac == KC - 1),
                )
            nc.vector.tensor_copy(out=v_sb, in_=v_ps)

            # w[b, m] = sum_k sT[k, b] * M_c[k, m] + 1 * v[m]
            w_ps = psum0.tile([B_LOC, D], FP, tag="w_ps")
            for kc in range(KC):
                nc.tensor.matmul(
                    w_ps,
                    lhsT=sT_sb[:, kc, :],
                    rhs=mc_sb[:, kc, :],
                    start=(kc == 0),
                    stop=False,
                )
            ones_1x8 = ones_1x128[:, :B_LOC]
            nc.tensor.matmul(w_ps, lhsT=ones_1x8, rhs=v_sb, start=False, stop=True)
            nc.vector.tensor_copy(out=w_sb16, in_=w_ps)  # cast fp32 -> fp16

        # relocate each w row to partition 0 (tiny SBUF->SBUF DMA on the
        # sync queue; PE matmul rhs must start at partition 0/32/64), then
        # broadcast it down the 128 partitions with a K=1 fp16 matmul and
        # cast during the ScalarE PSUM evacuation.
        for b in range(B_LOC):
            nc.sync.dma_start(out=w_rows16[:, b, :], in_=w_sb16[b : b + 1, :])
        with tc.tile_pool(name="psum1", bufs=2, space="PSUM") as psum1:
            for b in range(B_LOC):
                bc_ps = psum1.tile([P, D], FP, tag="bc_ps")
                nc.tensor.matmul(
                    bc_ps,
                    lhsT=ones16[0:1, :P],
                    rhs=w_rows16[:, b, :],
                )
                nc.scalar.copy(out=w_bc16[:, b, :], in_=bc_ps)

        # ---------------- stages 1+2: stream h ----------------
        with (
            tc.tile_pool(name="hpool", bufs=6) as hpool,
            tc.tile_pool(name="junk", bufs=2) as junk,
            tc.tile_pool(name="small", bufs=2) as small,
            tc.tile_pool(name="psum2", bufs=1, space="PSUM") as psum2,
        ):
            for b in range(B_LOC):
                for j in range(NSUP):
                    ht = hpool.tile([P, SUP, D], F16, tag="ht")
                    # SWDGE DMA with inline fp32->fp16 cast
                    nc.gpsimd.dma_start(
                        out=ht,
                        in_=h[b, j * SUP * P : (j + 1) * SUP * P, :].rearrange(
                            "(jt p) d -> p jt d", p=P
                        ),
                    )
                    for jt in range(SUP):
                        ti = j * SUP + jt
                        col = b * TI + ti
                        jd = junk.tile([P, D], F16, tag="jd")
                        # fused (h16 * w16) multiply + free-dim sum, fp32 accum
                        nc.vector.scalar_tensor_tensor(
                            out=jd,
                            in0=ht[:, jt, :],
                            scalar=1.0,
                            in1=w_bc16[:, b, :],
                            op0=ALU.mult,
                            op1=ALU.mult,
                            accum_out=e_all[:, col : col + 1],
                        )
                        if ti % 2 == 0:
                            ja = junk.tile([P, D], F16, tag="ja")
                            nc.scalar.activation(
                                out=ja,
                                in_=ht[:, jt, :],
                                func=AF.Copy,
                                accum_out=hs_all[:, col : col + 1],
                            )
                        else:
                            jb = junk.tile([P, D], F16, tag="jb")
                            nc.vector.scalar_tensor_tensor(
                                out=jb,
                                in0=ht[:, jt, :],
                                scalar=1.0,
                                in1=ones16,
                                op0=ALU.mult,
                                op1=ALU.mult,
                                accum_out=hs_all[:, col : col + 1],
                            )

                # ---- stage 2 for batch b: softmax over its 16 columns ----
                cols = slice(b * TI, (b + 1) * TI)

                colmax = small.tile([P, 1], FP, tag="colmax")
                nc.vector.tensor_reduce(
                    out=colmax, in_=e_all[:, cols], axis=mybir.AxisListType.X,
                    op=ALU.max,
                )
                cm_ps = psum2.tile([1, P], FP, tag="cm_ps")
                nc.tensor.transpose(cm_ps, in_=colmax, identity=ident)
                bmax = small.tile([1, 1], FP, tag="bmax")
                nc.vector.tensor_reduce(
                    out=bmax, in_=cm_ps, axis=mybir.AxisListType.X, op=ALU.max
                )
                # -bmax broadcast down the partitions: (-1s)^T @ bmax
                nb_ps = psum2.tile([P, 1], FP, tag="nb_ps")
                nc.tensor.matmul(nb_ps, lhsT=neg_1x128, rhs=bmax)
                nbmax = small.tile([P, 1], FP, tag="nbmax")
                nc.vector.tensor_copy(out=nbmax, in_=nb_ps)

                pscol = small.tile([P, 1], FP, tag="pscol")
                nc.scalar.activation(
                    out=exp_all[:, cols],
                    in_=e_all[:, cols],
                    func=AF.Exp,
                    bias=nbmax,
                    scale=1.0,
                    accum_out=pscol,
                )
                # total = sum_p pscol[p] via ones matmul
                es_ps = psum2.tile([1, 1], FP, tag="es_ps")
                nc.tensor.matmul(es_ps, lhsT=pscol, rhs=ones_128x1)
                rcp = small.tile([1, 1], FP, tag="rcp")
                nc.vector.reciprocal(out=rcp, in_=es_ps)
                rc_ps = psum2.tile([P, 1], FP, tag="rc_ps")
                nc.tensor.matmul(rc_ps, lhsT=ones_1x128, rhs=rcp)
                rcp_bc = small.tile([P, 1], FP, tag="rcp_bc")
                nc.vector.tensor_copy(out=rcp_bc, in_=rc_ps)

                cbuf = small.tile([P, TI], FP, tag="cbuf")
                nc.vector.tensor_tensor(
                    out=cbuf, in0=exp_all[:, cols], in1=hs_all[:, cols],
                    op=ALU.mult,
                )
                nc.vector.tensor_scalar_mul(out=cbuf, in0=cbuf, scalar1=rcp_bc)

                ct_ps = psum2.tile([TI, P], FP, tag="ct_ps")
                nc.tensor.transpose(ct_ps, in_=cbuf, identity=ident)
                ct_sb = small.tile([TI, P], FP, tag="ct_sb")
                nc.scalar.copy(out=ct_sb, in_=ct_ps)
                nc.sync.dma_start(
                    out=c_out[b, :].rearrange("(i p) -> i p", p=P), in_=ct_sb
                )


_CACHE = {}


def _build():
    if "nc" not in _CACHE:
        nc = bacc.Bacc(
            "TRN2", target_bir_lowering=False, debug=False, num_devices=N_CORES
        )
        with tile.TileContext(nc) as tc:
            _emit(nc, tc)
        nc.compile()
        _CACHE["nc"] = nc
    return _CACHE["nc"]


def kernel(s, h, phi_w, phi_b, psi_w, psi_b=None, **_unused):
    s = np.ascontiguousarray(np.asarray(s, dtype=np.float32))
    h = np.ascontiguousarray(np.asarray(h, dtype=np.float32))
    phi_w = np.ascontiguousarray(np.asarray(phi_w, dtype=np.float32))
    phi_b = np.ascontiguousarray(np.asarray(phi_b, dtype=np.float32))
    psi_w = np.ascontiguousarray(np.asarray(psi_w, dtype=np.float32))

    nc = _build()
    in_maps = [
        {
            "s": s[i * B_LOC : (i + 1) * B_LOC],
            "h": h[i * B_LOC : (i + 1) * B_LOC],
            "phi_w": phi_w,
            "phi_b": phi_b,
            "psi_w": psi_w,
        }
        for i in range(N_CORES)
    ]
    res = bass_utils.run_bass_kernel_spmd(nc, in_maps, core_ids=list(range(N_CORES)))
    return np.concatenate(
        [res.results[i]["c"] for i in range(N_CORES)], axis=0
    ).astype(np.float32)


# revision 10
# speedup vs baseline: 1.0215x; 1.0215x over previous
"""AttentionContext kernel for Trainium2, data-parallel over batch on 8 cores.

Reference computation (B=64, T=2048, D=512 everywhere):
    phi_s = s @ phi_w.T + phi_b                  # [B, D]
    psi_h = einsum('bth,ah->bta', h, psi_w) + psi_b
    e     = einsum('ba,bta->bt', phi_s, psi_h)   # [B, T]
    alpha = softmax(e, axis=-1)
    c     = alpha * h.sum(-1)                    # [B, T]

Algebraic restructuring:
    e[b,t] = (phi_s[b] @ psi_w) . h[b,t] + const(b); softmax drops const(b).
    w = s @ (phi_w.T @ psi_w) + phi_b @ psi_w    # [B, D], tiny, on PE

Streaming design (per core: 8 batches, 128 tiles of [128, 512]):
  * h is cast fp32->fp16 inside the SWDGE DMA (inline cast, HBM read traffic
    unchanged, SBUF writes halved). t-mapping t = p*16 + j makes every
    partition's slice of a batch a contiguous 32KB DRAM read (max DMA
    efficiency); softmax is permutation-invariant in t, and the output
    store needs no transpose in this layout.
  * e: all accumulate-capable DVE ops run at 1x, but plain fp16
    tensor_tensor runs at 2x (2 packed elems/port/cycle). So e is computed
    as product (TT mult against a stride-0-broadcast w) + log-fold tree of
    TT adds + one short segmented tensor_reduce: ~5.2us per 8 tiles vs
    ~6.1us for fused scalar_tensor_tensor accumulation.
  * hsum is split: 8 tiles/batch on ScalarE (activation-copy accumulate),
    8 tiles/batch as two fold chains on GpSimd (Pool tensor_tensor, ~2.5
    cyc/elem), final short reduces on DVE.
  * softmax: per-batch exp(e - colmax_p) with per-partition row max as the
    activation bias (negate=True reduce), then one batched cross-partition
    combine at the end: bmax via PE transpose + reduce, correction factor
    exp(colmax - bmax) folded into the final normalization multiply.
"""

import numpy as np

import concourse.bass as bass
import concourse.bacc as bacc
import concourse.tile as tile
from concourse import mybir
from concourse import bass_utils
from concourse.masks import make_identity

FP = mybir.dt.float32
F16 = mybir.dt.float16
ALU = mybir.AluOpType
AF = mybir.ActivationFunctionType

N_CORES = 8
B_LOC = 8          # batches per core
T = 2048
D = 512
P = 128
KC = D // P        # 4 contraction chunks of 128
NJ = T // P        # 16 t-tiles per batch (t = p*16 + j)

# per-batch hsum tile assignment: j in [0, SCAL_NJ) -> ScalarE;
# the rest in two GpSimd fold chains of GP_CHUNK tiles each
SCAL_NJ = 8
GP_CHUNK = 4


def _rep_ap(ap2, n):
    """[P, W] AP -> [P, n, W] view with stride-0 middle dim."""
    return bass.AP(
        tensor=ap2.tensor, offset=ap2.offset, ap=[ap2.ap[0], [0, n], ap2.ap[1]]
    )


def _emit(nc, tc):
    s = nc.dram_tensor("s", [B_LOC, D], FP, kind="ExternalInput").ap()
    h = nc.dram_tensor("h", [B_LOC, T, D], FP, kind="ExternalInput").ap()
    phi_w = nc.dram_tensor("phi_w", [D, D], FP, kind="ExternalInput").ap()
    phi_b = nc.dram_tensor("phi_b", [D], FP, kind="ExternalInput").ap()
    psi_w = nc.dram_tensor("psi_w", [D, D], FP, kind="ExternalInput").ap()
    c_out = nc.dram_tensor("c", [B_LOC, T], FP, kind="ExternalOutput").ap()

    with tc.tile_pool(name="consts", bufs=1) as consts:
        # ---------------- stage 0: combined weights ----------------
        phi_w_sb = consts.tile([P, KC, D], FP)   # [a % 128, a // 128, k]
        psi_w_sb = consts.tile([P, KC, D], FP)   # [a % 128, a // 128, m]
        for ac in range(KC):
            nc.sync.dma_start(
                out=phi_w_sb[:, ac, :], in_=phi_w[ac * P : (ac + 1) * P, :]
            )
            nc.sync.dma_start(
                out=psi_w_sb[:, ac, :], in_=psi_w[ac * P : (ac + 1) * P, :]
            )
        phi_b_sb = consts.tile([P, KC], FP)      # [a % 128, a // 128]
        nc.sync.dma_start(out=phi_b_sb, in_=phi_b.rearrange("(ac p) -> p ac", p=P))
        s_sb = consts.tile([B_LOC, D], FP)
        nc.sync.dma_start(out=s_sb, in_=s)

        ident = consts.tile([P, P], FP)
        make_identity(nc, ident)
        ones_1x128 = consts.tile([1, P], FP)
        nc.vector.memset(ones_1x128, 1.0)
        neg_1x128 = consts.tile([1, P], FP)
        nc.vector.memset(neg_1x128, -1.0)
        ones_128x1 = consts.tile([P, 1], FP)
        nc.vector.memset(ones_128x1, 1.0)
        ones16_1x128 = consts.tile([1, P], F16)
        nc.vector.memset(ones16_1x128, 1.0)

        # Warm the ACT exp table set early so the ~2.7us load overlaps.
        tiny = consts.tile([1, 1], FP)
        nc.vector.memset(tiny, 0.0)
        nc.scalar.activation(out=tiny, in_=tiny, func=AF.Exp)

        mc_sb = consts.tile([P, KC, D], FP)      # M_c[k, m], k = kc*128 + p
        v_sb = consts.tile([1, D], FP)           # v[m] = phi_b @ psi_w
        sT_sb = consts.tile([P, KC, B_LOC], FP)  # s.T[k, b]
        w_sb16 = consts.tile([B_LOC, D], F16)    # w[b, m] fp16
        w_rows16 = consts.tile([1, B_LOC, D], F16)  # each w row at partition 0
        w_bc16 = consts.tile([P, B_LOC, D], F16)  # w[b] broadcast down parts

        e_all = consts.tile([P, P], FP)          # e[p, b*16 + j], t = p*16+j
        hs_all = consts.tile([P, P], FP)         # hsum, same layout
        exp_all = consts.tile([P, P], FP)        # exp(e - colmax_p)
        ncm_all = consts.tile([P, B_LOC], FP)    # -colmax[p, b]
        pscol_all = consts.tile([P, B_LOC], FP)  # sum_j exp_all per (p, b)
        cmarg = consts.tile([P, B_LOC], FP)
        cmexp = consts.tile([P, B_LOC], FP)
        pscw = consts.tile([P, B_LOC], FP)
        nb_sb = consts.tile([P, B_LOC], FP)
        bmax_sb = consts.tile([B_LOC, 1], FP)
        bt_sb = consts.tile([1, B_LOC], FP)
        rcp_sb = consts.tile([B_LOC, 1], FP)
        rt_sb = consts.tile([1, B_LOC], FP)
        rb_sb = consts.tile([P, B_LOC], FP)
        wf_sb = consts.tile([P, B_LOC], FP)
        cbuf = consts.tile([P, P], FP)

        with tc.tile_pool(name="psum0", bufs=2, space="PSUM") as psum0:
            for kc in range(KC):
                st_ps = psum0.tile([P, B_LOC], FP, tag="st_ps")
                nc.tensor.transpose(
                    st_ps,
                    in_=s_sb[:, kc * P : (kc + 1) * P],
                    identity=ident[:B_LOC, :B_LOC],
                )
                nc.vector.tensor_copy(out=sT_sb[:, kc, :], in_=st_ps)

            # M_c[k, m] = sum_a phi_w[a, k] * psi_w[a, m]
            for kc in range(KC):
                mc_ps = psum0.tile([P, D], FP, tag="mc_ps")
                for ac in range(KC):
                    nc.tensor.matmul(
                        mc_ps,
                        lhsT=phi_w_sb[:, ac, kc * P : (kc + 1) * P],
                        rhs=psi_w_sb[:, ac, :],
                        start=(ac == 0),
                        stop=(ac == KC - 1),
                    )
                nc.vector.tensor_copy(out=mc_sb[:, kc, :], in_=mc_ps)

            # v[m] = sum_a phi_b[a] * psi_w[a, m]
            v_ps = psum0.tile([1, D], FP, tag="v_ps")
            for ac in range(KC):
                nc.tensor.matmul(
                    v_ps,
                    lhsT=phi_b_sb[:, ac : ac + 1],
                    rhs=psi_w_sb[:, ac, :],
                    start=(ac == 0),
                    stop=(ac == KC - 1),
                )
            nc.vector.tensor_copy(out=v_sb, in_=v_ps)

            # w[b, m] = sum_k sT[k, b] * M_c[k, m] + 1 * v[m]
            w_ps = psum0.tile([B_LOC, D], FP, tag="w_ps")
            for kc in range(KC):
                nc.tensor.matmul(
                    w_ps,
                    lhsT=sT_sb[:, kc, :],
                    rhs=mc_sb[:, kc, :],
                    start=(kc == 0),
                    stop=False,
                )
            nc.tensor.matmul(
                w_ps, lhsT=ones_1x128[:, :B_LOC], rhs=v_sb, start=False, stop=True
            )
            nc.vector.tensor_copy(out=w_sb16, in_=w_ps)  # cast fp32 -> fp16

        # relocate each w row to partition 0 (tiny SBUF->SBUF DMA on the
        # sync queue; PE matmul rhs must start at partition 0/32/64), then
        # broadcast it down the 128 partitions with a K=1 fp16 matmul and
        # cast during the ScalarE PSUM evacuation.
        for b in range(B_LOC):
            nc.sync.dma_start(out=w_rows16[:, b, :], in_=w_sb16[b : b + 1, :])
        with tc.tile_pool(name="psum1", bufs=2, space="PSUM") as psum1:
            for b in range(B_LOC):
                bc_ps = psum1.tile([P, D], FP, tag="bc_ps")
                nc.tensor.matmul(
                    bc_ps, lhsT=ones16_1x128, rhs=w_rows16[:, b, :]
                )
                nc.scalar.copy(out=w_bc16[:, b, :], in_=bc_ps)

        # ---------------- stage 1: stream h ----------------
        with (
            tc.tile_pool(name="hpool", bufs=4) as hpool,
            tc.tile_pool(name="ppool", bufs=2) as ppool,
            tc.tile_pool(name="fpool", bufs=2) as fpool,
            tc.tile_pool(name="gpool", bufs=2) as gpool,
            tc.tile_pool(name="junk", bufs=2) as junk,
            tc.tile_pool(name="psum2", bufs=1, space="PSUM") as psum2,
        ):
            for b in range(B_LOC):
                c0 = b * NJ
                ht = hpool.tile([P, NJ, D], F16, tag="ht")
                # SWDGE cast DMA; partition p reads 32KB contiguous
                nc.gpsimd.dma_start(
                    out=ht, in_=h[b].rearrange("(p j) d -> p j d", p=P)
                )

                # ---- e via product + fold tree, two half-batch chains ----
                w_rep = _rep_ap(w_bc16[:, b, :], NJ // 2)
                for v in range(2):
                    j0 = v * (NJ // 2)
                    prod = ppool.tile([P, NJ // 2, D], F16, tag="prod")
                    nc.vector.tensor_tensor(
                        out=prod, in0=ht[:, j0 : j0 + 8, :], in1=w_rep,
                        op=ALU.mult,
                    )
                    f1 = fpool.tile([P, NJ // 2, 256], F16, tag="f1")
                    nc.vector.tensor_tensor(
                        out=f1, in0=prod[:, :, 0:256], in1=prod[:, :, 256:512],
                        op=ALU.add,
                    )
                    f2 = fpool.tile([P, NJ // 2, 128], F16, tag="f2")
                    nc.vector.tensor_tensor(
                        out=f2, in0=f1[:, :, 0:128], in1=f1[:, :, 128:256],
                        op=ALU.add,
                    )
                    f3 = fpool.tile([P, NJ // 2, 64], F16, tag="f3")
                    nc.vector.tensor_tensor(
                        out=f3, in0=f2[:, :, 0:64], in1=f2[:, :, 64:128],
                        op=ALU.add,
                    )
                    f4 = fpool.tile([P, NJ // 2, 32], F16, tag="f4")
                    nc.vector.tensor_tensor(
                        out=f4, in0=f3[:, :, 0:32], in1=f3[:, :, 32:64],
                        op=ALU.add,
                    )
                    nc.vector.tensor_reduce(
                        out=e_all[:, c0 + j0 : c0 + j0 + 8], in_=f4,
                        axis=mybir.AxisListType.X, op=ALU.add,
                    )

                # ---- hsum: ScalarE tiles ----
                for j in range(SCAL_NJ):
                    col = c0 + j
                    ja = junk.tile([P, D], F16, tag="ja")
                    nc.scalar.activation(
                        out=ja, in_=ht[:, j, :], func=AF.Copy,
                        accum_out=hs_all[:, col : col + 1],
                    )
                # ---- hsum: GpSimd fold chains ----
                for j0 in range(SCAL_NJ, NJ, GP_CHUNK):
                    g1 = gpool.tile([P, GP_CHUNK, 256], F16, tag="g1")
                    nc.gpsimd.tensor_tensor(
                        out=g1, in0=ht[:, j0 : j0 + GP_CHUNK, 0:256],
                        in1=ht[:, j0 : j0 + GP_CHUNK, 256:512], op=ALU.add,
                    )
                    g2 = gpool.tile([P, GP_CHUNK, 128], F16, tag="g2")
                    nc.gpsimd.tensor_tensor(
                        out=g2, in0=g1[:, :, 0:128], in1=g1[:, :, 128:256],
                        op=ALU.add,
                    )
                    g3 = gpool.tile([P, GP_CHUNK, 64], F16, tag="g3")
                    nc.gpsimd.tensor_tensor(
                        out=g3, in0=g2[:, :, 0:64], in1=g2[:, :, 64:128],
                        op=ALU.add,
                    )
                    g4 = gpool.tile([P, GP_CHUNK, 32], F16, tag="g4")
                    nc.gpsimd.tensor_tensor(
                        out=g4, in0=g3[:, :, 0:32], in1=g3[:, :, 32:64],
                        op=ALU.add,
                    )
                    nc.vector.tensor_reduce(
                        out=hs_all[:, c0 + j0 : c0 + j0 + GP_CHUNK], in_=g4,
                        axis=mybir.AxisListType.X, op=ALU.add,
                    )

                # ---- per-batch: row max and exp(e - colmax_p) ----
                nc.vector.tensor_reduce(
                    out=ncm_all[:, b : b + 1], in_=e_all[:, c0 : c0 + NJ],
                    axis=mybir.AxisListType.X, op=ALU.max, negate=True,
                )
                nc.scalar.activation(
                    out=exp_all[:, c0 : c0 + NJ], in_=e_all[:, c0 : c0 + NJ],
                    func=AF.Exp, bias=ncm_all[:, b : b + 1], scale=1.0,
                    accum_out=pscol_all[:, b : b + 1],
                )

            # ---------------- stage 2: batched softmax combine ----------------
            # bmax[b] = max_p colmax[p, b]; ncm = -colmax
            cmT_ps = psum2.tile([B_LOC, P], FP, tag="cmT_ps")
            nc.tensor.transpose(cmT_ps, in_=ncm_all, identity=ident)
            nc.vector.tensor_reduce(
                out=bmax_sb, in_=cmT_ps, axis=mybir.AxisListType.X,
                op=ALU.min, negate=True,
            )
            bt_ps = psum2.tile([1, B_LOC], FP, tag="bt_ps")
            nc.tensor.transpose(bt_ps, in_=bmax_sb, identity=ident[:B_LOC, :B_LOC])
            nc.vector.tensor_copy(out=bt_sb, in_=bt_ps)
            # -bmax broadcast down partitions
            nb_ps = psum2.tile([P, B_LOC], FP, tag="nb_ps")
            nc.tensor.matmul(nb_ps, lhsT=neg_1x128, rhs=bt_sb)
            nc.vector.tensor_copy(out=nb_sb, in_=nb_ps)
            # cmarg = colmax - bmax = (-ncm) + (-bmax) ... = -(ncm) + nb
            nc.vector.tensor_tensor(
                out=cmarg, in0=nb_sb, in1=ncm_all, op=ALU.subtract
            )
            nc.scalar.activation(out=cmexp, in_=cmarg, func=AF.Exp)
            nc.vector.tensor_tensor(
                out=pscw, in0=pscol_all, in1=cmexp, op=ALU.mult
            )
            # Z[b] = sum_p pscw[p, b]
            z_ps = psum2.tile([B_LOC, 1], FP, tag="z_ps")
            nc.tensor.matmul(z_ps, lhsT=pscw, rhs=ones_128x1)
            nc.vector.reciprocal(out=rcp_sb, in_=z_ps)
            rt_ps = psum2.tile([1, B_LOC], FP, tag="rt_ps")
            nc.tensor.transpose(rt_ps, in_=rcp_sb, identity=ident[:B_LOC, :B_LOC])
            nc.vector.tensor_copy(out=rt_sb, in_=rt_ps)
            rb_ps = psum2.tile([P, B_LOC], FP, tag="rb_ps")
            nc.tensor.matmul(rb_ps, lhsT=ones_1x128, rhs=rt_sb)
            nc.vector.tensor_copy(out=rb_sb, in_=rb_ps)
            nc.vector.tensor_tensor(out=wf_sb, in0=cmexp, in1=rb_sb, op=ALU.mult)

            # c = exp_all * hs_all * wf[p, b]  (wf broadcast over j)
            nc.vector.tensor_tensor(
                out=cbuf, in0=exp_all, in1=hs_all, op=ALU.mult
            )
            wf_rep = bass.AP(
                tensor=wf_sb.tensor, offset=wf_sb.offset,
                ap=[wf_sb.ap[0], wf_sb.ap[1], [0, NJ]],
            )
            cbuf3 = bass.AP(
                tensor=cbuf.tensor, offset=cbuf.offset,
                ap=[cbuf.ap[0], [NJ, B_LOC], [1, NJ]],
            )
            nc.vector.tensor_tensor(out=cbuf3, in0=cbuf3, in1=wf_rep, op=ALU.mult)
            # c[b, p*16 + j] = cbuf[p, b*16 + j]
            nc.sync.dma_start(
                out=c_out.rearrange("b (p j) -> p b j", p=P), in_=cbuf
            )


_CACHE = {}


def _build():
    if "nc" not in _CACHE:
        nc = bacc.Bacc(
            "TRN2", target_bir_lowering=False, debug=False, num_devices=N_CORES
        )
        with tile.TileContext(nc) as tc:
            _emit(nc, tc)
        nc.compile()
        _CACHE["nc"] = nc
    return _CACHE["nc"]


def kernel(s, h, phi_w, phi_b, psi_w, psi_b=None, **_unused):
    s = np.ascontiguousarray(np.asarray(s, dtype=np.float32))
    h = np.ascontiguousarray(np.asarray(h, dtype=np.float32))
    phi_w = np.ascontiguousarray(np.asarray(phi_w, dtype=np.float32))
    phi_b = np.ascontiguousarray(np.asarray(phi_b, dtype=np.float32))
    psi_w = np.ascontiguousarray(np.asarray(psi_w, dtype=np.float32))

    nc = _build()
    in_maps = [
        {
            "s": s[i * B_LOC : (i + 1) * B_LOC],
            "h": h[i * B_LOC : (i + 1) * B_LOC],
            "phi_w": phi_w,
            "phi_b": phi_b,
            "psi_w": psi_w,
        }
        for i in range(N_CORES)
    ]
    res = bass_utils.run_bass_kernel_spmd(nc, in_maps, core_ids=list(range(N_CORES)))
    return np.concatenate(
        [res.results[i]["c"] for i in range(N_CORES)], axis=0
    ).astype(np.float32)


# revision 17
# speedup vs baseline: 1.0435x; 1.0215x over previous
"""AttentionContext kernel for Trainium2, data-parallel over batch on 8 cores.

Reference computation (B=64, T=2048, D=512 everywhere):
    phi_s = s @ phi_w.T + phi_b                  # [B, D]
    psi_h = einsum('bth,ah->bta', h, psi_w) + psi_b
    e     = einsum('ba,bta->bt', phi_s, psi_h)   # [B, T]
    alpha = softmax(e, axis=-1)
    c     = alpha * h.sum(-1)                    # [B, T]

Algebraic restructuring:
    e[b,t] = (phi_s[b] @ psi_w) . h[b,t] + const(b); softmax drops const(b).
    w = s @ (phi_w.T @ psi_w) + phi_b @ psi_w    # [B, D], tiny, on PE

Streaming design (per core: 8 batches, 128 tiles of [128, 512]):
  * h is cast fp32->fp16 inside the SWDGE DMA (inline cast, HBM read traffic
    unchanged, SBUF writes halved). t-mapping t = p*16 + j makes every
    partition's slice of a batch a contiguous 32KB DRAM read (max DMA
    efficiency); softmax is permutation-invariant in t, and the output
    store needs no transpose in this layout.
  * e: all accumulate-capable DVE ops run at 1x, but plain fp16
    tensor_tensor runs at 2x (2 packed elems/port/cycle). So e is computed
    as product (TT mult against a stride-0-broadcast w) + log-fold tree of
    TT adds + one short segmented tensor_reduce: ~5.2us per 8 tiles vs
    ~6.1us for fused scalar_tensor_tensor accumulation.
  * hsum is split: 8 tiles/batch on ScalarE (activation-copy accumulate),
    8 tiles/batch as two fold chains on GpSimd (Pool tensor_tensor, ~2.5
    cyc/elem), final short reduces on DVE.
  * softmax: per-batch exp(e - colmax_p) with per-partition row max as the
    activation bias (negate=True reduce), then one batched cross-partition
    combine at the end: bmax via PE transpose + reduce, correction factor
    exp(colmax - bmax) folded into the final normalization multiply.
"""

import numpy as np

import concourse.bass as bass
import concourse.bacc as bacc
import concourse.tile as tile
from concourse import mybir
from concourse import bass_utils
from concourse.masks import make_identity

FP = mybir.dt.float32
F16 = mybir.dt.float16
ALU = mybir.AluOpType
AF = mybir.ActivationFunctionType

N_CORES = 8
B_LOC = 8          # batches per core
T = 2048
D = 512
P = 128
KC = D // P        # 4 contraction chunks of 128
NJ = T // P        # 16 t-tiles per batch (t = p*16 + j)

# per-batch hsum tile assignment: j in [0, SCAL_NJ) -> ScalarE
# (activation-copy accumulate); j in [SCAL_NJ, NJ) -> one DVE fold chain.
# GpSimd compute is intentionally unused: Pool shares an SBUF port pair with
# the DVE under an exclusive per-instruction lock, and this kernel's DVE
# stream is almost entirely two-input tensor_tensor ops that hold that pair.
SCAL_NJ = 12


def _rep_ap(ap2, n):
    """[P, W] AP -> [P, n, W] view with stride-0 middle dim."""
    return bass.AP(
        tensor=ap2.tensor, offset=ap2.offset, ap=[ap2.ap[0], [0, n], ap2.ap[1]]
    )


def _emit(nc, tc):
    s = nc.dram_tensor("s", [B_LOC, D], FP, kind="ExternalInput").ap()
    h = nc.dram_tensor("h", [B_LOC, T, D], FP, kind="ExternalInput").ap()
    phi_w = nc.dram_tensor("phi_w", [D, D], FP, kind="ExternalInput").ap()
    phi_b = nc.dram_tensor("phi_b", [D], FP, kind="ExternalInput").ap()
    psi_w = nc.dram_tensor("psi_w", [D, D], FP, kind="ExternalInput").ap()
    c_out = nc.dram_tensor("c", [B_LOC, T], FP, kind="ExternalOutput").ap()

    with tc.tile_pool(name="consts", bufs=1) as consts:
        # ---------------- stage 0: combined weights ----------------
        phi_w_sb = consts.tile([P, KC, D], FP)   # [a % 128, a // 128, k]
        psi_w_sb = consts.tile([P, KC, D], FP)   # [a % 128, a // 128, m]
        s_sb = consts.tile([B_LOC, D], FP)
        nc.sync.dma_start(out=s_sb, in_=s)
        nc.sync.dma_start(
            out=phi_w_sb, in_=phi_w.rearrange("(ac p) k -> p ac k", p=P)
        )
        nc.sync.dma_start(
            out=psi_w_sb, in_=psi_w.rearrange("(ac p) k -> p ac k", p=P)
        )
        phi_b_sb = consts.tile([P, KC], FP)      # [a % 128, a // 128]
        nc.sync.dma_start(out=phi_b_sb, in_=phi_b.rearrange("(ac p) -> p ac", p=P))

        ident = consts.tile([P, P], FP)
        make_identity(nc, ident)
        ones_1x128 = consts.tile([1, P], FP)
        nc.vector.memset(ones_1x128, 1.0)
        neg_1x128 = consts.tile([1, P], FP)
        nc.vector.memset(neg_1x128, -1.0)
        ones_128x1 = consts.tile([P, 1], FP)
        nc.vector.memset(ones_128x1, 1.0)
        ones16_1x128 = consts.tile([1, P], F16)
        nc.vector.memset(ones16_1x128, 1.0)

        # Warm the ACT exp table set early so the ~2.7us load overlaps.
        tiny = consts.tile([1, 1], FP)
        nc.vector.memset(tiny, 0.0)
        nc.scalar.activation(out=tiny, in_=tiny, func=AF.Exp)

        mc_sb = consts.tile([P, KC, D], FP)      # M_c[k, m], k = kc*128 + p
        v_sb = consts.tile([1, D], FP)           # v[m] = phi_b @ psi_w
        sT_sb = consts.tile([P, KC, B_LOC], FP)  # s.T[k, b]
        w_sb16 = consts.tile([B_LOC, D], F16)    # w[b, m] fp16
        w_rows16 = consts.tile([1, B_LOC, D], F16)  # each w row at partition 0
        w_bc16 = consts.tile([P, B_LOC, D], F16)  # w[b] broadcast down parts

        e_all = consts.tile([P, P], FP)          # e[p, b*16 + j], t = p*16+j
        hs_all = consts.tile([P, P], FP)         # hsum, same layout
        exp_all = consts.tile([P, P], FP)        # exp(e - colmax_p)
        ncm_all = consts.tile([P, B_LOC], FP)    # -colmax[p, b]
        pscol_all = consts.tile([P, B_LOC], FP)  # sum_j exp_all per (p, b)
        cmarg = consts.tile([P, B_LOC], FP)
        cmexp = consts.tile([P, B_LOC], FP)
        pscw = consts.tile([P, B_LOC], FP)
        nb_sb = consts.tile([P, B_LOC], FP)
        bmax_sb = consts.tile([B_LOC, 1], FP)
        bt_sb = consts.tile([1, B_LOC], FP)
        rcp_sb = consts.tile([B_LOC, 1], FP)
        rt_sb = consts.tile([1, B_LOC], FP)
        rb_sb = consts.tile([P, B_LOC], FP)
        wf_sb = consts.tile([P, B_LOC], FP)
        cbuf = consts.tile([P, P], FP)

        # HAM warm-up: ~3.5us of back-to-back PE activity (identity
        # transposes, results discarded) flips the PE clock gate to 8/8
        # before the real stage-0 chain, halving its matmul durations.
        with tc.tile_pool(name="psumw", bufs=2, space="PSUM") as psumw:
            for _ in range(10):
                warm_ps = psumw.tile([P, P], FP, tag="warm_ps")
                nc.tensor.transpose(warm_ps, in_=ident, identity=ident)

        with tc.tile_pool(name="psum0", bufs=2, space="PSUM") as psum0:
            for kc in range(KC):
                st_ps = psum0.tile([P, B_LOC], FP, tag="st_ps")
                nc.tensor.transpose(
                    st_ps,
                    in_=s_sb[:, kc * P : (kc + 1) * P],
                    identity=ident[:B_LOC, :B_LOC],
                )
                nc.vector.tensor_copy(out=sT_sb[:, kc, :], in_=st_ps)

            # M_c[k, m] = sum_a phi_w[a, k] * psi_w[a, m]
            for kc in range(KC):
                mc_ps = psum0.tile([P, D], FP, tag="mc_ps")
                for ac in range(KC):
                    nc.tensor.matmul(
                        mc_ps,
                        lhsT=phi_w_sb[:, ac, kc * P : (kc + 1) * P],
                        rhs=psi_w_sb[:, ac, :],
                        start=(ac == 0),
                        stop=(ac == KC - 1),
                    )
                nc.vector.tensor_copy(out=mc_sb[:, kc, :], in_=mc_ps)

            # v[m] = sum_a phi_b[a] * psi_w[a, m]
            v_ps = psum0.tile([1, D], FP, tag="v_ps")
            for ac in range(KC):
                nc.tensor.matmul(
                    v_ps,
                    lhsT=phi_b_sb[:, ac : ac + 1],
                    rhs=psi_w_sb[:, ac, :],
                    start=(ac == 0),
                    stop=(ac == KC - 1),
                )
            nc.vector.tensor_copy(out=v_sb, in_=v_ps)

            # w[b, m] = sum_k sT[k, b] * M_c[k, m] + 1 * v[m]
            w_ps = psum0.tile([B_LOC, D], FP, tag="w_ps")
            for kc in range(KC):
                nc.tensor.matmul(
                    w_ps,
                    lhsT=sT_sb[:, kc, :],
                    rhs=mc_sb[:, kc, :],
                    start=(kc == 0),
                    stop=False,
                )
            nc.tensor.matmul(
                w_ps, lhsT=ones_1x128[:, :B_LOC], rhs=v_sb, start=False, stop=True
            )
            nc.vector.tensor_copy(out=w_sb16, in_=w_ps)  # cast fp32 -> fp16

        # relocate each w row to partition 0 (tiny SBUF->SBUF DMA on the
        # sync queue; PE matmul rhs must start at partition 0/32/64), then
        # broadcast it down the 128 partitions with a K=1 fp16 matmul and
        # cast during the ScalarE PSUM evacuation.
        for b in range(B_LOC):
            nc.sync.dma_start(out=w_rows16[:, b, :], in_=w_sb16[b : b + 1, :])
        with tc.tile_pool(name="psum1", bufs=2, space="PSUM") as psum1:
            for b in range(B_LOC):
                bc_ps = psum1.tile([P, D], FP, tag="bc_ps")
                nc.tensor.matmul(
                    bc_ps, lhsT=ones16_1x128, rhs=w_rows16[:, b, :]
                )
                # evacuate on DVE: ScalarE's strict FIFO would order these
                # behind early hsum activations, starving the e-products
                nc.vector.tensor_copy(out=w_bc16[:, b, :], in_=bc_ps)

        # ---------------- stage 1: stream h ----------------
        with (
            tc.tile_pool(name="hpool", bufs=4) as hpool,
            tc.tile_pool(name="ppool", bufs=2) as ppool,
            tc.tile_pool(name="fpool", bufs=2) as fpool,
            tc.tile_pool(name="gpool", bufs=2) as gpool,
            tc.tile_pool(name="junk", bufs=2) as junk,
            tc.tile_pool(name="psum2", bufs=1, space="PSUM") as psum2,
        ):
            GN = NJ - SCAL_NJ  # tiles per batch in the DVE hsum fold chain
            for b in range(B_LOC):
                c0 = b * NJ
                ht = hpool.tile([P, NJ, D], F16, tag="ht")
                # SWDGE cast DMA; partition p reads 32KB contiguous.
                # Batch 0 loads in quarters so compute can start ~9us earlier.
                if b == 0:
                    q = NJ // 4
                    h3 = h[b].rearrange("(p j) d -> p j d", p=P)
                    for v in range(4):
                        nc.gpsimd.dma_start(
                            out=ht[:, v * q : (v + 1) * q, :],
                            in_=h3[:, v * q : (v + 1) * q, :],
                        )
                else:
                    nc.gpsimd.dma_start(
                        out=ht, in_=h[b].rearrange("(p j) d -> p j d", p=P)
                    )

                # ---- hsum: ScalarE tiles (emitted first: no dep on w) ----
                for j in range(SCAL_NJ):
                    col = c0 + j
                    ja = junk.tile([P, D], F16, tag="ja")
                    nc.scalar.activation(
                        out=ja, in_=ht[:, j, :], func=AF.Copy,
                        accum_out=hs_all[:, col : col + 1],
                    )
                # ---- hsum: DVE fold chain over the remaining tiles ----
                g1 = gpool.tile([P, GN, 256], F16, tag="g1")
                nc.vector.tensor_tensor(
                    out=g1, in0=ht[:, SCAL_NJ:, 0:256],
                    in1=ht[:, SCAL_NJ:, 256:512], op=ALU.add,
                )
                g2 = gpool.tile([P, GN, 128], F16, tag="g2")
                nc.vector.tensor_tensor(
                    out=g2, in0=g1[:, :, 0:128], in1=g1[:, :, 128:256],
                    op=ALU.add,
                )
                g3 = gpool.tile([P, GN, 64], F16, tag="g3")
                nc.vector.tensor_tensor(
                    out=g3, in0=g2[:, :, 0:64], in1=g2[:, :, 64:128],
                    op=ALU.add,
                )
                g4 = gpool.tile([P, GN, 32], F16, tag="g4")
                nc.vector.tensor_tensor(
                    out=g4, in0=g3[:, :, 0:32], in1=g3[:, :, 32:64],
                    op=ALU.add,
                )
                nc.vector.tensor_reduce(
                    out=hs_all[:, c0 + SCAL_NJ : c0 + NJ], in_=g4,
                    axis=mybir.AxisListType.X, op=ALU.add,
                )

                # ---- e via product + full-batch fold tree ----
                w_rep = _rep_ap(w_bc16[:, b, :], NJ)
                prod = ppool.tile([P, NJ, D], F16, tag="prod")
                nc.vector.tensor_tensor(
                    out=prod, in0=ht, in1=w_rep, op=ALU.mult
                )
                f1 = fpool.tile([P, NJ, 256], F16, tag="f1")
                nc.vector.tensor_tensor(
                    out=f1, in0=prod[:, :, 0:256], in1=prod[:, :, 256:512],
                    op=ALU.add,
                )
                f2 = fpool.tile([P, NJ, 128], F16, tag="f2")
                nc.vector.tensor_tensor(
                    out=f2, in0=f1[:, :, 0:128], in1=f1[:, :, 128:256],
                    op=ALU.add,
                )
                f3 = fpool.tile([P, NJ, 64], F16, tag="f3")
                nc.vector.tensor_tensor(
                    out=f3, in0=f2[:, :, 0:64], in1=f2[:, :, 64:128],
                    op=ALU.add,
                )
                f4 = fpool.tile([P, NJ, 32], F16, tag="f4")
                nc.vector.tensor_tensor(
                    out=f4, in0=f3[:, :, 0:32], in1=f3[:, :, 32:64],
                    op=ALU.add,
                )
                nc.vector.tensor_reduce(
                    out=e_all[:, c0 : c0 + NJ], in_=f4,
                    axis=mybir.AxisListType.X, op=ALU.add,
                )

                # ---- per-batch: row max and exp(e - colmax_p) ----
                nc.vector.tensor_reduce(
                    out=ncm_all[:, b : b + 1], in_=e_all[:, c0 : c0 + NJ],
                    axis=mybir.AxisListType.X, op=ALU.max, negate=True,
                )
                nc.scalar.activation(
                    out=exp_all[:, c0 : c0 + NJ], in_=e_all[:, c0 : c0 + NJ],
                    func=AF.Exp, bias=ncm_all[:, b : b + 1], scale=1.0,
                    accum_out=pscol_all[:, b : b + 1],
                )

            # ---------------- stage 2: batched softmax combine ----------------
            # bmax[b] = max_p colmax[p, b]; ncm = -colmax
            cmT_ps = psum2.tile([B_LOC, P], FP, tag="cmT_ps")
            nc.tensor.transpose(cmT_ps, in_=ncm_all, identity=ident)
            nc.vector.tensor_reduce(
                out=bmax_sb, in_=cmT_ps, axis=mybir.AxisListType.X,
                op=ALU.min, negate=True,
            )
            bt_ps = psum2.tile([1, B_LOC], FP, tag="bt_ps")
            nc.tensor.transpose(bt_ps, in_=bmax_sb, identity=ident[:B_LOC, :B_LOC])
            nc.vector.tensor_copy(out=bt_sb, in_=bt_ps)
            # -bmax broadcast down partitions
            nb_ps = psum2.tile([P, B_LOC], FP, tag="nb_ps")
            nc.tensor.matmul(nb_ps, lhsT=neg_1x128, rhs=bt_sb)
            nc.vector.tensor_copy(out=nb_sb, in_=nb_ps)
            # cmarg = colmax - bmax = (-ncm) + (-bmax) ... = -(ncm) + nb
            nc.vector.tensor_tensor(
                out=cmarg, in0=nb_sb, in1=ncm_all, op=ALU.subtract
            )
            nc.scalar.activation(out=cmexp, in_=cmarg, func=AF.Exp)
            nc.vector.tensor_tensor(
                out=pscw, in0=pscol_all, in1=cmexp, op=ALU.mult
            )
            # Z[b] = sum_p pscw[p, b]
            z_ps = psum2.tile([B_LOC, 1], FP, tag="z_ps")
            nc.tensor.matmul(z_ps, lhsT=pscw, rhs=ones_128x1)
            nc.vector.reciprocal(out=rcp_sb, in_=z_ps)
            rt_ps = psum2.tile([1, B_LOC], FP, tag="rt_ps")
            nc.tensor.transpose(rt_ps, in_=rcp_sb, identity=ident[:B_LOC, :B_LOC])
            nc.vector.tensor_copy(out=rt_sb, in_=rt_ps)
            rb_ps = psum2.tile([P, B_LOC], FP, tag="rb_ps")
            nc.tensor.matmul(rb_ps, lhsT=ones_1x128, rhs=rt_sb)
            nc.vector.tensor_copy(out=rb_sb, in_=rb_ps)
            nc.vector.tensor_tensor(out=wf_sb, in0=cmexp, in1=rb_sb, op=ALU.mult)

            # c = exp_all * hs_all * wf[p, b]  (wf broadcast over j)
            nc.vector.tensor_tensor(
                out=cbuf, in0=exp_all, in1=hs_all, op=ALU.mult
            )
            wf_rep = bass.AP(
                tensor=wf_sb.tensor, offset=wf_sb.offset,
                ap=[wf_sb.ap[0], wf_sb.ap[1], [0, NJ]],
            )
            cbuf3 = bass.AP(
                tensor=cbuf.tensor, offset=cbuf.offset,
                ap=[cbuf.ap[0], [NJ, B_LOC], [1, NJ]],
            )
            nc.vector.tensor_tensor(out=cbuf3, in0=cbuf3, in1=wf_rep, op=ALU.mult)
            # c[b, p*16 + j] = cbuf[p, b*16 + j]
            nc.sync.dma_start(
                out=c_out.rearrange("b (p j) -> p b j", p=P), in_=cbuf
            )


_CACHE = {}


def _build():
    if "nc" not in _CACHE:
        nc = bacc.Bacc(
            "TRN2", target_bir_lowering=False, debug=False, num_devices=N_CORES
        )
        with tile.TileContext(nc) as tc:
            _emit(nc, tc)
        nc.compile()
        _CACHE["nc"] = nc
    return _CACHE["nc"]


def kernel(s, h, phi_w, phi_b, psi_w, psi_b=None, **_unused):
    s = np.ascontiguousarray(np.asarray(s, dtype=np.float32))
    h = np.ascontiguousarray(np.asarray(h, dtype=np.float32))
    phi_w = np.ascontiguousarray(np.asarray(phi_w, dtype=np.float32))
    phi_b = np.ascontiguousarray(np.asarray(phi_b, dtype=np.float32))
    psi_w = np.ascontiguousarray(np.asarray(psi_w, dtype=np.float32))

    nc = _build()
    in_maps = [
        {
            "s": s[i * B_LOC : (i + 1) * B_LOC],
            "h": h[i * B_LOC : (i + 1) * B_LOC],
            "phi_w": phi_w,
            "phi_b": phi_b,
            "psi_w": psi_w,
        }
        for i in range(N_CORES)
    ]
    res = bass_utils.run_bass_kernel_spmd(nc, in_maps, core_ids=list(range(N_CORES)))
    return np.concatenate(
        [res.results[i]["c"] for i in range(N_CORES)], axis=0
    ).astype(np.float32)


# revision 20
# speedup vs baseline: 1.2675x; 1.2147x over previous
"""AttentionContext kernel for Trainium2, data-parallel over batch on 8 cores.

Reference computation (B=64, T=2048, D=512 everywhere):
    phi_s = s @ phi_w.T + phi_b                  # [B, D]
    psi_h = einsum('bth,ah->bta', h, psi_w) + psi_b
    e     = einsum('ba,bta->bt', phi_s, psi_h)   # [B, T]
    alpha = softmax(e, axis=-1)
    c     = alpha * h.sum(-1)                    # [B, T]

Algebraic restructuring:
    e[b,t] = (phi_s[b] @ psi_w) . h[b,t] + const(b); softmax drops const(b).
    w = s @ (phi_w.T @ psi_w) + phi_b @ psi_w    # [B, D], tiny, on PE

Streaming design (per core: 8 batches, 128 tiles of [128, 512]):
  * All DRAM traffic rides one SWDGE (Pool) queue: small fp16-cast weight
    loads first (HWDGE weight loads starve behind a saturated SWDGE h
    stream at the shared SDMA engines), then the h stream. h is cast
    fp32->fp16 inline in the DMA; the t-mapping t = p*16 + j makes each
    partition's slice of a batch one contiguous 32KB DRAM read. Softmax is
    permutation-invariant in t and the output store needs no transpose.
  * e: accumulate-capable DVE ops run at 1x, but plain fp16 tensor_tensor
    runs at 2x. e is computed as one whole-batch product (TT mult against
    a stride-0-broadcast w) + a log-fold tree of TT adds + one short
    segmented tensor_reduce (~10us per 16-tile batch).
  * hsum: 12 tiles/batch on ScalarE activation-copy-accumulate, 4 via a
    DVE fold chain. GpSimd compute is intentionally unused: Pool shares an
    SBUF port pair with the DVE under an exclusive per-instruction lock
    and this DVE stream is nearly all two-input tensor_tensor ops.
  * stage 0 runs in fp16 on a HAM-warmed PE so w is broadcast (K=1 matmul
    per batch + DVE PSUM evacuation) before the first batch finishes
    loading. First and last batch load in quarters to compress ramp/tail;
    the last batch's e runs as quarter fold-trees behind its quarters.
  * softmax: per-batch exp(e - colmax_p) with the row max as activation
    bias (negate=True reduce), then one batched cross-partition combine:
    bmax via PE transpose + reduce, correction exp(colmax - bmax) folded
    into the final normalization multiply, single 64KB store.
"""

import numpy as np

import concourse.bass as bass
import concourse.bacc as bacc
import concourse.tile as tile
from concourse import mybir
from concourse import bass_utils
from concourse.masks import make_identity

FP = mybir.dt.float32
F16 = mybir.dt.float16
ALU = mybir.AluOpType
AF = mybir.ActivationFunctionType

N_CORES = 8
B_LOC = 8          # batches per core
T = 2048
D = 512
P = 128
KC = D // P        # 4 contraction chunks of 128
NJ = T // P        # 16 t-tiles per batch (t = p*16 + j)

SCAL_NJ = 12       # hsum tiles per batch on ScalarE (rest: DVE fold chain)


def _rep_ap(ap2, n):
    """[P, W] AP -> [P, n, W] view with stride-0 middle dim."""
    return bass.AP(
        tensor=ap2.tensor, offset=ap2.offset, ap=[ap2.ap[0], [0, n], ap2.ap[1]]
    )


def _emit(nc, tc):
    s = nc.dram_tensor("s", [B_LOC, D], FP, kind="ExternalInput").ap()
    h = nc.dram_tensor("h", [B_LOC, T, D], FP, kind="ExternalInput").ap()
    phi_w = nc.dram_tensor("phi_w", [D, D], FP, kind="ExternalInput").ap()
    phi_b = nc.dram_tensor("phi_b", [D], FP, kind="ExternalInput").ap()
    psi_w = nc.dram_tensor("psi_w", [D, D], FP, kind="ExternalInput").ap()
    c_out = nc.dram_tensor("c", [B_LOC, T], FP, kind="ExternalOutput").ap()

    with tc.tile_pool(name="consts", bufs=1) as consts:
        # ------- stage 0 inputs: fp16 cast loads, FIRST on the Pool queue ----
        s_sb = consts.tile([B_LOC, D], FP)
        nc.gpsimd.dma_start(out=s_sb, in_=s)
        phi16_sb = consts.tile([P, KC, D], F16)  # [a % 128, a // 128, k]
        nc.gpsimd.dma_start(
            out=phi16_sb, in_=phi_w.rearrange("(ac p) k -> p ac k", p=P)
        )
        psi16_sb = consts.tile([P, KC, D], F16)  # [a % 128, a // 128, m]
        nc.gpsimd.dma_start(
            out=psi16_sb, in_=psi_w.rearrange("(ac p) k -> p ac k", p=P)
        )
        phi_b16 = consts.tile([P, KC], F16)      # [a % 128, a // 128]
        nc.gpsimd.dma_start(
            out=phi_b16, in_=phi_b.rearrange("(ac p) -> p ac", p=P)
        )

        ident = consts.tile([P, P], FP)
        make_identity(nc, ident)
        ident16 = consts.tile([P, P], F16)
        nc.vector.tensor_copy(out=ident16, in_=ident)
        ones_1x128 = consts.tile([1, P], FP)
        nc.vector.memset(ones_1x128, 1.0)
        neg_1x128 = consts.tile([1, P], FP)
        nc.vector.memset(neg_1x128, -1.0)
        ones_128x1 = consts.tile([P, 1], FP)
        nc.vector.memset(ones_128x1, 1.0)
        ones16_1x128 = consts.tile([1, P], F16)
        nc.vector.memset(ones16_1x128, 1.0)

        # Warm the ACT exp table set early so the ~2.7us load overlaps.
        tiny = consts.tile([1, 1], FP)
        nc.vector.memset(tiny, 0.0)
        nc.scalar.activation(out=tiny, in_=tiny, func=AF.Exp)

        mc_sb = consts.tile([P, KC, D], F16)     # M_c[k, m], k = kc*128 + p
        v_sb = consts.tile([1, D], F16)          # v[m] = phi_b @ psi_w
        sT_sb = consts.tile([P, KC, B_LOC], F16)  # s.T[k, b]
        w_sb16 = consts.tile([B_LOC, D], F16)    # w[b, m] fp16
        w_rows16 = consts.tile([1, B_LOC, D], F16)  # each w row at partition 0
        w_bc16 = consts.tile([P, B_LOC, D], F16)  # w[b] broadcast down parts

        e_all = consts.tile([P, P], FP)          # e[p, b*16 + j], t = p*16+j
        hs_all = consts.tile([P, P], FP)         # hsum, same layout
        exp_all = consts.tile([P, P], FP)        # exp(e - colmax_p)
        ncm_all = consts.tile([P, B_LOC], FP)    # -colmax[p, b]
        pscol_all = consts.tile([P, B_LOC], FP)  # sum_j exp_all per (p, b)
        cmarg = consts.tile([P, B_LOC], FP)
        cmexp = consts.tile([P, B_LOC], FP)
        pscw = consts.tile([P, B_LOC], FP)
        nb_sb = consts.tile([P, B_LOC], FP)
        bmax_sb = consts.tile([B_LOC, 1], FP)
        bt_sb = consts.tile([1, B_LOC], FP)
        rcp_sb = consts.tile([B_LOC, 1], FP)
        rt_sb = consts.tile([1, B_LOC], FP)
        rb_sb = consts.tile([P, B_LOC], FP)
        wf_sb = consts.tile([P, B_LOC], FP)
        cbuf = consts.tile([P, P], FP)

        # HAM warm-up: ~3us of back-to-back PE activity (identity transposes,
        # results discarded) flips the PE clock gate to 8/8 before stage 0.
        with tc.tile_pool(name="psumw", bufs=2, space="PSUM") as psumw:
            for _ in range(10):
                warm_ps = psumw.tile([P, P], FP, tag="warm_ps")
                nc.tensor.transpose(warm_ps, in_=ident, identity=ident)

        with tc.tile_pool(name="psum0", bufs=2, space="PSUM") as psum0:
            for kc in range(KC):
                st_ps = psum0.tile([P, B_LOC], FP, tag="st_ps")
                nc.tensor.transpose(
                    st_ps,
                    in_=s_sb[:, kc * P : (kc + 1) * P],
                    identity=ident[:B_LOC, :B_LOC],
                )
                nc.vector.tensor_copy(out=sT_sb[:, kc, :], in_=st_ps)

            # M_c[k, m] = sum_a phi_w[a, k] * psi_w[a, m]   (fp16 operands)
            for kc in range(KC):
                mc_ps = psum0.tile([P, D], FP, tag="mc_ps")
                for ac in range(KC):
                    nc.tensor.matmul(
                        mc_ps,
                        lhsT=phi16_sb[:, ac, kc * P : (kc + 1) * P],
                        rhs=psi16_sb[:, ac, :],
                        start=(ac == 0),
                        stop=(ac == KC - 1),
                    )
                nc.vector.tensor_copy(out=mc_sb[:, kc, :], in_=mc_ps)

            # v[m] = sum_a phi_b[a] * psi_w[a, m]
            v_ps = psum0.tile([1, D], FP, tag="v_ps")
            for ac in range(KC):
                nc.tensor.matmul(
                    v_ps,
                    lhsT=phi_b16[:, ac : ac + 1],
                    rhs=psi16_sb[:, ac, :],
                    start=(ac == 0),
                    stop=(ac == KC - 1),
                )
            nc.vector.tensor_copy(out=v_sb, in_=v_ps)

            # w[b, m] = sum_k sT[k, b] * M_c[k, m] + 1 * v[m]
            w_ps = psum0.tile([B_LOC, D], FP, tag="w_ps")
            for kc in range(KC):
                nc.tensor.matmul(
                    w_ps,
                    lhsT=sT_sb[:, kc, :],
                    rhs=mc_sb[:, kc, :],
                    start=(kc == 0),
                    stop=False,
                )
            nc.tensor.matmul(
                w_ps, lhsT=ones16_1x128[:, :B_LOC], rhs=v_sb,
                start=False, stop=True,
            )
            nc.vector.tensor_copy(out=w_sb16, in_=w_ps)  # cast fp32 -> fp16

        # relocate each w row to partition 0 (tiny SBUF->SBUF DMA on the
        # sync queue; PE matmul rhs must start at partition 0/32/64), then
        # broadcast it down the 128 partitions with a K=1 fp16 matmul.
        for b in range(B_LOC):
            nc.sync.dma_start(out=w_rows16[:, b, :], in_=w_sb16[b : b + 1, :])
        with tc.tile_pool(name="psum1", bufs=2, space="PSUM") as psum1:
            for b in range(B_LOC):
                bc_ps = psum1.tile([P, D], FP, tag="bc_ps")
                nc.tensor.matmul(
                    bc_ps, lhsT=ones16_1x128, rhs=w_rows16[:, b, :]
                )
                # evacuate on DVE: ScalarE's strict FIFO would order these
                # behind early hsum activations, starving the e-products
                nc.vector.tensor_copy(out=w_bc16[:, b, :], in_=bc_ps)

        # ---------------- stage 1: stream h ----------------
        with (
            tc.tile_pool(name="hpool", bufs=3) as hpool,
            tc.tile_pool(name="ppool", bufs=2) as ppool,
            tc.tile_pool(name="fpool", bufs=2) as fpool,
            tc.tile_pool(name="gpool", bufs=2) as gpool,
            tc.tile_pool(name="junk", bufs=2) as junk,
            tc.tile_pool(name="psum2", bufs=1, space="PSUM") as psum2,
        ):
            GN = NJ - SCAL_NJ  # tiles per batch in the DVE hsum fold chain
            QN = NJ // 4

            def e_tree(src3, w_rep_n, ecols, tag_sfx=""):
                """product + fold tree + segmented reduce -> e_all[:, ecols]"""
                n = src3.shape[1]
                prod = ppool.tile([P, n, D], F16, tag="prod" + tag_sfx)
                nc.vector.tensor_tensor(out=prod, in0=src3, in1=w_rep_n,
                                        op=ALU.mult)
                f1 = fpool.tile([P, n, 256], F16, tag="f1" + tag_sfx)
                nc.vector.tensor_tensor(
                    out=f1, in0=prod[:, :, 0:256], in1=prod[:, :, 256:512],
                    op=ALU.add)
                f2 = fpool.tile([P, n, 128], F16, tag="f2" + tag_sfx)
                nc.vector.tensor_tensor(
                    out=f2, in0=f1[:, :, 0:128], in1=f1[:, :, 128:256],
                    op=ALU.add)
                f3 = fpool.tile([P, n, 64], F16, tag="f3" + tag_sfx)
                nc.vector.tensor_tensor(
                    out=f3, in0=f2[:, :, 0:64], in1=f2[:, :, 64:128],
                    op=ALU.add)
                f4 = fpool.tile([P, n, 32], F16, tag="f4" + tag_sfx)
                nc.vector.tensor_tensor(
                    out=f4, in0=f3[:, :, 0:32], in1=f3[:, :, 32:64],
                    op=ALU.add)
                nc.vector.tensor_reduce(
                    out=e_all[:, ecols], in_=f4,
                    axis=mybir.AxisListType.X, op=ALU.add)

            for b in range(B_LOC):
                c0 = b * NJ
                last = b == B_LOC - 1
                ht = hpool.tile([P, NJ, D], F16, tag="ht")
                # SWDGE cast DMA; partition p reads 32KB contiguous.
                # First/last batch load in quarters to compress ramp/tail.
                if b == 0 or last:
                    h3 = h[b].rearrange("(p j) d -> p j d", p=P)
                    for v in range(4):
                        nc.gpsimd.dma_start(
                            out=ht[:, v * QN : (v + 1) * QN, :],
                            in_=h3[:, v * QN : (v + 1) * QN, :],
                        )
                else:
                    nc.gpsimd.dma_start(
                        out=ht, in_=h[b].rearrange("(p j) d -> p j d", p=P)
                    )

                # ---- hsum: ScalarE tiles (no dependency on w) ----
                scal_nj = NJ if last else SCAL_NJ
                for j in range(scal_nj):
                    col = c0 + j
                    ja = junk.tile([P, D], F16, tag="ja")
                    nc.scalar.activation(
                        out=ja, in_=ht[:, j, :], func=AF.Copy,
                        accum_out=hs_all[:, col : col + 1],
                    )
                # ---- hsum: DVE fold chain over the remaining tiles ----
                if not last:
                    g1 = gpool.tile([P, GN, 256], F16, tag="g1")
                    nc.vector.tensor_tensor(
                        out=g1, in0=ht[:, SCAL_NJ:, 0:256],
                        in1=ht[:, SCAL_NJ:, 256:512], op=ALU.add)
                    g2 = gpool.tile([P, GN, 128], F16, tag="g2")
                    nc.vector.tensor_tensor(
                        out=g2, in0=g1[:, :, 0:128], in1=g1[:, :, 128:256],
                        op=ALU.add)
                    g3 = gpool.tile([P, GN, 64], F16, tag="g3")
                    nc.vector.tensor_tensor(
                        out=g3, in0=g2[:, :, 0:64], in1=g2[:, :, 64:128],
                        op=ALU.add)
                    g4 = gpool.tile([P, GN, 32], F16, tag="g4")
                    nc.vector.tensor_tensor(
                        out=g4, in0=g3[:, :, 0:32], in1=g3[:, :, 32:64],
                        op=ALU.add)
                    nc.vector.tensor_reduce(
                        out=hs_all[:, c0 + SCAL_NJ : c0 + NJ], in_=g4,
                        axis=mybir.AxisListType.X, op=ALU.add)

                # ---- e: whole-batch tree; last batch as quarter trees ----
                if last:
                    wq = _rep_ap(w_bc16[:, b, :], QN)
                    for v in range(4):
                        e_tree(
                            ht[:, v * QN : (v + 1) * QN, :], wq,
                            slice(c0 + v * QN, c0 + (v + 1) * QN), "q",
                        )
                else:
                    e_tree(ht, _rep_ap(w_bc16[:, b, :], NJ),
                           slice(c0, c0 + NJ))

                # ---- per-batch: row max and exp(e - colmax_p) ----
                nc.vector.tensor_reduce(
                    out=ncm_all[:, b : b + 1], in_=e_all[:, c0 : c0 + NJ],
                    axis=mybir.AxisListType.X, op=ALU.max, negate=True,
                )
                nc.scalar.activation(
                    out=exp_all[:, c0 : c0 + NJ], in_=e_all[:, c0 : c0 + NJ],
                    func=AF.Exp, bias=ncm_all[:, b : b + 1], scale=1.0,
                    accum_out=pscol_all[:, b : b + 1],
                )

            # ---------------- stage 2: batched softmax combine ----------------
            # bmax[b] = max_p colmax[p, b]; ncm = -colmax
            cmT_ps = psum2.tile([B_LOC, P], FP, tag="cmT_ps")
            nc.tensor.transpose(cmT_ps, in_=ncm_all, identity=ident)
            nc.vector.tensor_reduce(
                out=bmax_sb, in_=cmT_ps, axis=mybir.AxisListType.X,
                op=ALU.min, negate=True,
            )
            bt_ps = psum2.tile([1, B_LOC], FP, tag="bt_ps")
            nc.tensor.transpose(bt_ps, in_=bmax_sb, identity=ident[:B_LOC, :B_LOC])
            nc.vector.tensor_copy(out=bt_sb, in_=bt_ps)
            # -bmax broadcast down partitions
            nb_ps = psum2.tile([P, B_LOC], FP, tag="nb_ps")
            nc.tensor.matmul(nb_ps, lhsT=neg_1x128, rhs=bt_sb)
            nc.vector.tensor_copy(out=nb_sb, in_=nb_ps)
            # cmarg = colmax - bmax = nb - ncm
            nc.vector.tensor_tensor(
                out=cmarg, in0=nb_sb, in1=ncm_all, op=ALU.subtract
            )
            nc.scalar.activation(out=cmexp, in_=cmarg, func=AF.Exp)
            nc.vector.tensor_tensor(
                out=pscw, in0=pscol_all, in1=cmexp, op=ALU.mult
            )
            # Z[b] = sum_p pscw[p, b]
            z_ps = psum2.tile([B_LOC, 1], FP, tag="z_ps")
            nc.tensor.matmul(z_ps, lhsT=pscw, rhs=ones_128x1)
            nc.vector.reciprocal(out=rcp_sb, in_=z_ps)
            rt_ps = psum2.tile([1, B_LOC], FP, tag="rt_ps")
            nc.tensor.transpose(rt_ps, in_=rcp_sb, identity=ident[:B_LOC, :B_LOC])
            nc.vector.tensor_copy(out=rt_sb, in_=rt_ps)
            rb_ps = psum2.tile([P, B_LOC], FP, tag="rb_ps")
            nc.tensor.matmul(rb_ps, lhsT=ones_1x128, rhs=rt_sb)
            nc.vector.tensor_copy(out=rb_sb, in_=rb_ps)
            nc.vector.tensor_tensor(out=wf_sb, in0=cmexp, in1=rb_sb, op=ALU.mult)

            # c = exp_all * hs_all * wf[p, b]  (wf broadcast over j)
            nc.vector.tensor_tensor(
                out=cbuf, in0=exp_all, in1=hs_all, op=ALU.mult
            )
            wf_rep = bass.AP(
                tensor=wf_sb.tensor, offset=wf_sb.offset,
                ap=[wf_sb.ap[0], wf_sb.ap[1], [0, NJ]],
            )
            cbuf3 = bass.AP(
                tensor=cbuf.tensor, offset=cbuf.offset,
                ap=[cbuf.ap[0], [NJ, B_LOC], [1, NJ]],
            )
            nc.vector.tensor_tensor(out=cbuf3, in0=cbuf3, in1=wf_rep, op=ALU.mult)
            # c[b, p*16 + j] = cbuf[p, b*16 + j]
            nc.sync.dma_start(
                out=c_out.rearrange("b (p j) -> p b j", p=P), in_=cbuf
            )


_CACHE = {}


def _build():
    if "nc" not in _CACHE:
        nc = bacc.Bacc(
            "TRN2", target_bir_lowering=False, debug=False, num_devices=N_CORES
        )
        with tile.TileContext(nc) as tc:
            _emit(nc, tc)
        nc.compile()
        _CACHE["nc"] = nc
    return _CACHE["nc"]


def kernel(s, h, phi_w, phi_b, psi_w, psi_b=None, **_unused):
    s = np.ascontiguousarray(np.asarray(s, dtype=np.float32))
    h = np.ascontiguousarray(np.asarray(h, dtype=np.float32))
    phi_w = np.ascontiguousarray(np.asarray(phi_w, dtype=np.float32))
    phi_b = np.ascontiguousarray(np.asarray(phi_b, dtype=np.float32))
    psi_w = np.ascontiguousarray(np.asarray(psi_w, dtype=np.float32))

    nc = _build()
    in_maps = [
        {
            "s": s[i * B_LOC : (i + 1) * B_LOC],
            "h": h[i * B_LOC : (i + 1) * B_LOC],
            "phi_w": phi_w,
            "phi_b": phi_b,
            "psi_w": psi_w,
        }
        for i in range(N_CORES)
    ]
    res = bass_utils.run_bass_kernel_spmd(nc, in_maps, core_ids=list(range(N_CORES)))
    return np.concatenate(
        [res.results[i]["c"] for i in range(N_CORES)], axis=0
    ).astype(np.float32)


# revision 22
# speedup vs baseline: 1.4529x; 1.1463x over previous
"""AttentionContext kernel for Trainium2, data-parallel over batch on 8 cores.

Reference computation (B=64, T=2048, D=512 everywhere):
    phi_s = s @ phi_w.T + phi_b                  # [B, D]
    psi_h = einsum('bth,ah->bta', h, psi_w) + psi_b
    e     = einsum('ba,bta->bt', phi_s, psi_h)   # [B, T]
    alpha = softmax(e, axis=-1)
    c     = alpha * h.sum(-1)                    # [B, T]

Algebraic restructuring:
    e[b,t] = (phi_s[b] @ psi_w) . h[b,t] + const(b); softmax drops const(b).
    w = s @ (phi_w.T @ psi_w) + phi_b @ psi_w    # [B, D], tiny, on PE

Streaming design (per core: 8 batches, 128 tiles of [128, 512]):
  * All DRAM traffic rides one SWDGE (Pool) queue: small fp16-cast weight
    loads first (HWDGE weight loads starve behind a saturated SWDGE h
    stream at the shared SDMA engines), then the h stream. h is cast
    fp32->fp16 inline in the DMA; the t-mapping t = p*16 + j makes each
    partition's slice of a batch one contiguous 32KB DRAM read. Softmax is
    permutation-invariant in t and the output store needs no transpose.
  * The free-dim reductions (e = sum_d h*w and hsum = sum_d h) ride the
    TensorEngine: with a stationary fp16 identity, 16 accumulating
    matmuls per batch fold [128, 16, 512] -> PSUM [128, 16, 32] (each
    rhs chunk is an identity-copy accumulated into the same PSUM bank,
    fp32). One segmented DVE tensor_reduce finishes 32 -> 1 straight out
    of PSUM. The DVE only computes the h*w product (fp16 tensor_tensor at
    2 elem/cycle against a stride-0-broadcast w), in halves interleaved
    with the PE folds to keep the PE clock-gate (HAM) warm.
  * stage 0 runs in fp16 on a HAM-warmed PE so w is broadcast (K=1 matmul
    per batch + PSUM evacuation) before the first batch finishes loading.
    The last batch loads in quarter tiles and folds per quarter to keep
    the post-stream tail short.
  * softmax: per-batch exp(e - colmax_p) on ScalarE with the row max as
    activation bias (negate=True reduce), then one batched cross-partition
    combine: bmax via PE transpose + reduce, correction exp(colmax - bmax)
    folded into the final normalization multiply, single 64KB store.
"""

import numpy as np

import concourse.bass as bass
import concourse.bacc as bacc
import concourse.tile as tile
from concourse import mybir
from concourse import bass_utils
from concourse.masks import make_identity

FP = mybir.dt.float32
F16 = mybir.dt.float16
ALU = mybir.AluOpType
AF = mybir.ActivationFunctionType

N_CORES = 8
B_LOC = 8          # batches per core
T = 2048
D = 512
P = 128
KC = D // P        # 4 contraction chunks of 128
NJ = T // P        # 16 t-tiles per batch (t = p*16 + j)
QW = 32            # PE fold output width: 512 -> QW via D//QW matmuls


def _rep_ap(ap2, n):
    """[P, W] AP -> [P, n, W] view with stride-0 middle dim."""
    return bass.AP(
        tensor=ap2.tensor, offset=ap2.offset, ap=[ap2.ap[0], [0, n], ap2.ap[1]]
    )


def _emit(nc, tc):
    s = nc.dram_tensor("s", [B_LOC, D], FP, kind="ExternalInput").ap()
    h = nc.dram_tensor("h", [B_LOC, T, D], FP, kind="ExternalInput").ap()
    phi_w = nc.dram_tensor("phi_w", [D, D], FP, kind="ExternalInput").ap()
    phi_b = nc.dram_tensor("phi_b", [D], FP, kind="ExternalInput").ap()
    psi_w = nc.dram_tensor("psi_w", [D, D], FP, kind="ExternalInput").ap()
    c_out = nc.dram_tensor("c", [B_LOC, T], FP, kind="ExternalOutput").ap()

    with tc.tile_pool(name="consts", bufs=1) as consts:
        # ------- stage 0 inputs: fp16 cast loads, FIRST on the Pool queue ----
        s_sb = consts.tile([B_LOC, D], FP)
        nc.gpsimd.dma_start(out=s_sb, in_=s)
        phi16_sb = consts.tile([P, KC, D], F16)  # [a % 128, a // 128, k]
        nc.gpsimd.dma_start(
            out=phi16_sb, in_=phi_w.rearrange("(ac p) k -> p ac k", p=P)
        )
        psi16_sb = consts.tile([P, KC, D], F16)  # [a % 128, a // 128, m]
        nc.gpsimd.dma_start(
            out=psi16_sb, in_=psi_w.rearrange("(ac p) k -> p ac k", p=P)
        )
        phi_b16 = consts.tile([P, KC], F16)      # [a % 128, a // 128]
        nc.gpsimd.dma_start(
            out=phi_b16, in_=phi_b.rearrange("(ac p) -> p ac", p=P)
        )

        ident = consts.tile([P, P], FP)
        make_identity(nc, ident)
        ident16 = consts.tile([P, P], F16)
        nc.vector.tensor_copy(out=ident16, in_=ident)
        ones_1x128 = consts.tile([1, P], FP)
        nc.vector.memset(ones_1x128, 1.0)
        neg_1x128 = consts.tile([1, P], FP)
        nc.vector.memset(neg_1x128, -1.0)
        ones_128x1 = consts.tile([P, 1], FP)
        nc.vector.memset(ones_128x1, 1.0)
        ones16_1x128 = consts.tile([1, P], F16)
        nc.vector.memset(ones16_1x128, 1.0)

        # Warm the ACT exp table set early so the ~2.7us load overlaps.
        tiny = consts.tile([1, 1], FP)
        nc.vector.memset(tiny, 0.0)
        nc.scalar.activation(out=tiny, in_=tiny, func=AF.Exp)

        mc_sb = consts.tile([P, KC, D], F16)     # M_c[k, m], k = kc*128 + p
        v_sb = consts.tile([1, D], F16)          # v[m] = phi_b @ psi_w
        sT_sb = consts.tile([P, KC, B_LOC], F16)  # s.T[k, b]
        w_sb16 = consts.tile([B_LOC, D], F16)    # w[b, m] fp16
        w_rows16 = consts.tile([1, B_LOC, D], F16)  # each w row at partition 0
        w_bc16 = consts.tile([P, B_LOC, D], F16)  # w[b] broadcast down parts

        e_all = consts.tile([P, P], FP)          # e[p, b*16 + j], t = p*16+j
        hs_all = consts.tile([P, P], FP)         # hsum, same layout
        exp_all = consts.tile([P, P], FP)        # exp(e - colmax_p)
        ncm_all = consts.tile([P, B_LOC], FP)    # -colmax[p, b]
        pscol_all = consts.tile([P, B_LOC], FP)  # sum_j exp_all per (p, b)
        cmarg = consts.tile([P, B_LOC], FP)
        cmexp = consts.tile([P, B_LOC], FP)
        pscw = consts.tile([P, B_LOC], FP)
        nb_sb = consts.tile([P, B_LOC], FP)
        bmax_sb = consts.tile([B_LOC, 1], FP)
        bt_sb = consts.tile([1, B_LOC], FP)
        rcp_sb = consts.tile([B_LOC, 1], FP)
        rt_sb = consts.tile([1, B_LOC], FP)
        rb_sb = consts.tile([P, B_LOC], FP)
        wf_sb = consts.tile([P, B_LOC], FP)
        cbuf = consts.tile([P, P], FP)

        # HAM warm-up: ~3us of back-to-back PE activity (identity transposes,
        # results discarded) flips the PE clock gate to 8/8 before stage 0.
        with tc.tile_pool(name="psumw", bufs=2, space="PSUM") as psumw:
            for _ in range(10):
                warm_ps = psumw.tile([P, P], FP, tag="warm_ps")
                nc.tensor.transpose(warm_ps, in_=ident, identity=ident)

        with tc.tile_pool(name="psum0", bufs=2, space="PSUM") as psum0:
            for kc in range(KC):
                st_ps = psum0.tile([P, B_LOC], FP, tag="st_ps")
                nc.tensor.transpose(
                    st_ps,
                    in_=s_sb[:, kc * P : (kc + 1) * P],
                    identity=ident[:B_LOC, :B_LOC],
                )
                nc.vector.tensor_copy(out=sT_sb[:, kc, :], in_=st_ps)

            # M_c[k, m] = sum_a phi_w[a, k] * psi_w[a, m]   (fp16 operands)
            for kc in range(KC):
                mc_ps = psum0.tile([P, D], FP, tag="mc_ps")
                for ac in range(KC):
                    nc.tensor.matmul(
                        mc_ps,
                        lhsT=phi16_sb[:, ac, kc * P : (kc + 1) * P],
                        rhs=psi16_sb[:, ac, :],
                        start=(ac == 0),
                        stop=(ac == KC - 1),
                    )
                nc.vector.tensor_copy(out=mc_sb[:, kc, :], in_=mc_ps)

            # v[m] = sum_a phi_b[a] * psi_w[a, m]
            v_ps = psum0.tile([1, D], FP, tag="v_ps")
            for ac in range(KC):
                nc.tensor.matmul(
                    v_ps,
                    lhsT=phi_b16[:, ac : ac + 1],
                    rhs=psi16_sb[:, ac, :],
                    start=(ac == 0),
                    stop=(ac == KC - 1),
                )
            nc.vector.tensor_copy(out=v_sb, in_=v_ps)

            # w[b, m] = sum_k sT[k, b] * M_c[k, m] + 1 * v[m]
            w_ps = psum0.tile([B_LOC, D], FP, tag="w_ps")
            for kc in range(KC):
                nc.tensor.matmul(
                    w_ps,
                    lhsT=sT_sb[:, kc, :],
                    rhs=mc_sb[:, kc, :],
                    start=(kc == 0),
                    stop=False,
                )
            nc.tensor.matmul(
                w_ps, lhsT=ones16_1x128[:, :B_LOC], rhs=v_sb,
                start=False, stop=True,
            )
            nc.vector.tensor_copy(out=w_sb16, in_=w_ps)  # cast fp32 -> fp16

        # relocate each w row to partition 0 (tiny SBUF->SBUF DMA on the
        # sync queue; PE matmul rhs must start at partition 0/32/64), then
        # broadcast it down the 128 partitions with a K=1 fp16 matmul.
        for b in range(B_LOC):
            nc.sync.dma_start(out=w_rows16[:, b, :], in_=w_sb16[b : b + 1, :])
        with tc.tile_pool(name="psum1", bufs=2, space="PSUM") as psum1:
            for b in range(B_LOC):
                bc_ps = psum1.tile([P, D], FP, tag="bc_ps")
                nc.tensor.matmul(
                    bc_ps, lhsT=ones16_1x128, rhs=w_rows16[:, b, :]
                )
                # evacuate on DVE: ScalarE's strict FIFO would order these
                # behind early hsum activations, starving the e-products
                nc.vector.tensor_copy(out=w_bc16[:, b, :], in_=bc_ps)

        # ---------------- stage 1: stream h ----------------
        with (
            tc.tile_pool(name="hpool", bufs=4) as hpool,
            tc.tile_pool(name="qpool", bufs=4) as qpool,
            tc.tile_pool(name="ppool", bufs=2) as ppool,
            tc.tile_pool(name="psum3", bufs=2, space="PSUM") as psum3,
        ):
            QN = NJ // 4

            def pe_fold(src3, n, ps_tag):
                """[P, n, D] -> PSUM [P, n, QW] via accumulating identity
                matmuls; returns the PSUM tile."""
                ps = psum3.tile([P, n, QW], FP, tag=ps_tag)
                nchunk = D // QW
                for ci in range(nchunk):
                    nc.tensor.matmul(
                        ps, lhsT=ident16,
                        rhs=src3[:, :, ci * QW : (ci + 1) * QW],
                        start=(ci == 0), stop=(ci == nchunk - 1),
                    )
                return ps

            def do_block(src3, b, j0, n):
                """hsum + e for tiles [j0, j0+n) of batch b from src3."""
                cols = slice(b * NJ + j0, b * NJ + j0 + n)
                hs_ps = pe_fold(src3, n, f"hs_ps{n}")
                nc.vector.tensor_reduce(
                    out=hs_all[:, cols], in_=hs_ps,
                    axis=mybir.AxisListType.X, op=ALU.add,
                )
                # product in halves, each followed by its PE fold, so the
                # PE never idles past the ~3.4us HAM re-throttle window
                prod = ppool.tile([P, n, D], F16, tag=f"prod{n}")
                nh = max(n // 2, 1)
                for v in range(n // nh):
                    nc.vector.tensor_tensor(
                        out=prod[:, v * nh : (v + 1) * nh, :],
                        in0=src3[:, v * nh : (v + 1) * nh, :],
                        in1=_rep_ap(w_bc16[:, b, :], nh),
                        op=ALU.mult,
                    )
                e_ps = pe_fold(prod, n, f"e_ps{n}")
                nc.vector.tensor_reduce(
                    out=e_all[:, cols], in_=e_ps,
                    axis=mybir.AxisListType.X, op=ALU.add,
                )

            for b in range(B_LOC):
                c0 = b * NJ
                last = b == B_LOC - 1
                if last:
                    # quarter tiles: precise deps so the tail work starts
                    # as each quarter lands, not after the full batch
                    h3 = h[b].rearrange("(p j) d -> p j d", p=P)
                    for v in range(4):
                        hq = qpool.tile([P, QN, D], F16, tag=f"hq{v}")
                        nc.gpsimd.dma_start(
                            out=hq, in_=h3[:, v * QN : (v + 1) * QN, :]
                        )
                        do_block(hq, b, v * QN, QN)
                else:
                    ht = hpool.tile([P, NJ, D], F16, tag="ht")
                    nc.gpsimd.dma_start(
                        out=ht, in_=h[b].rearrange("(p j) d -> p j d", p=P)
                    )
                    do_block(ht, b, 0, NJ)

                # ---- per-batch: row max and exp(e - colmax_p) ----
                nc.vector.tensor_reduce(
                    out=ncm_all[:, b : b + 1], in_=e_all[:, c0 : c0 + NJ],
                    axis=mybir.AxisListType.X, op=ALU.max, negate=True,
                )
                nc.scalar.activation(
                    out=exp_all[:, c0 : c0 + NJ], in_=e_all[:, c0 : c0 + NJ],
                    func=AF.Exp, bias=ncm_all[:, b : b + 1], scale=1.0,
                    accum_out=pscol_all[:, b : b + 1],
                )

        # ---------------- stage 2: batched softmax combine ----------------
        with tc.tile_pool(name="psum2", bufs=1, space="PSUM") as psum2:
            # bmax[b] = max_p colmax[p, b]; ncm = -colmax
            cmT_ps = psum2.tile([B_LOC, P], FP, tag="cmT_ps")
            nc.tensor.transpose(cmT_ps, in_=ncm_all, identity=ident)
            nc.vector.tensor_reduce(
                out=bmax_sb, in_=cmT_ps, axis=mybir.AxisListType.X,
                op=ALU.min, negate=True,
            )
            bt_ps = psum2.tile([1, B_LOC], FP, tag="bt_ps")
            nc.tensor.transpose(
                bt_ps, in_=bmax_sb, identity=ident[:B_LOC, :B_LOC]
            )
            nc.vector.tensor_copy(out=bt_sb, in_=bt_ps)
            # -bmax broadcast down partitions
            nb_ps = psum2.tile([P, B_LOC], FP, tag="nb_ps")
            nc.tensor.matmul(nb_ps, lhsT=neg_1x128, rhs=bt_sb)
            nc.vector.tensor_copy(out=nb_sb, in_=nb_ps)
            # cmarg = colmax - bmax = nb - ncm
            nc.vector.tensor_tensor(
                out=cmarg, in0=nb_sb, in1=ncm_all, op=ALU.subtract
            )
            nc.scalar.activation(out=cmexp, in_=cmarg, func=AF.Exp)
            nc.vector.tensor_tensor(
                out=pscw, in0=pscol_all, in1=cmexp, op=ALU.mult
            )
            # Z[b] = sum_p pscw[p, b]
            z_ps = psum2.tile([B_LOC, 1], FP, tag="z_ps")
            nc.tensor.matmul(z_ps, lhsT=pscw, rhs=ones_128x1)
            nc.vector.reciprocal(out=rcp_sb, in_=z_ps)
            rt_ps = psum2.tile([1, B_LOC], FP, tag="rt_ps")
            nc.tensor.transpose(
                rt_ps, in_=rcp_sb, identity=ident[:B_LOC, :B_LOC]
            )
            nc.vector.tensor_copy(out=rt_sb, in_=rt_ps)
            rb_ps = psum2.tile([P, B_LOC], FP, tag="rb_ps")
            nc.tensor.matmul(rb_ps, lhsT=ones_1x128, rhs=rt_sb)
            nc.vector.tensor_copy(out=rb_sb, in_=rb_ps)
            nc.vector.tensor_tensor(out=wf_sb, in0=cmexp, in1=rb_sb, op=ALU.mult)

            # c = exp_all * hs_all * wf[p, b]  (wf broadcast over j)
            nc.vector.tensor_tensor(
                out=cbuf, in0=exp_all, in1=hs_all, op=ALU.mult
            )
            wf_rep = bass.AP(
                tensor=wf_sb.tensor, offset=wf_sb.offset,
                ap=[wf_sb.ap[0], wf_sb.ap[1], [0, NJ]],
            )
            cbuf3 = bass.AP(
                tensor=cbuf.tensor, offset=cbuf.offset,
                ap=[cbuf.ap[0], [NJ, B_LOC], [1, NJ]],
            )
            nc.vector.tensor_tensor(
                out=cbuf3, in0=cbuf3, in1=wf_rep, op=ALU.mult
            )
            # c[b, p*16 + j] = cbuf[p, b*16 + j]
            nc.sync.dma_start(
                out=c_out.rearrange("b (p j) -> p b j", p=P), in_=cbuf
            )


_CACHE = {}


def _build():
    if "nc" not in _CACHE:
        nc = bacc.Bacc(
            "TRN2", target_bir_lowering=False, debug=False, num_devices=N_CORES
        )
        with tile.TileContext(nc) as tc:
            _emit(nc, tc)
        nc.compile()
        _CACHE["nc"] = nc
    return _CACHE["nc"]


def kernel(s, h, phi_w, phi_b, psi_w, psi_b=None, **_unused):
    s = np.ascontiguousarray(np.asarray(s, dtype=np.float32))
    h = np.ascontiguousarray(np.asarray(h, dtype=np.float32))
    phi_w = np.ascontiguousarray(np.asarray(phi_w, dtype=np.float32))
    phi_b = np.ascontiguousarray(np.asarray(phi_b, dtype=np.float32))
    psi_w = np.ascontiguousarray(np.asarray(psi_w, dtype=np.float32))

    nc = _build()
    in_maps = [
        {
            "s": s[i * B_LOC : (i + 1) * B_LOC],
            "h": h[i * B_LOC : (i + 1) * B_LOC],
            "phi_w": phi_w,
            "phi_b": phi_b,
            "psi_w": psi_w,
        }
        for i in range(N_CORES)
    ]
    res = bass_utils.run_bass_kernel_spmd(nc, in_maps, core_ids=list(range(N_CORES)))
    return np.concatenate(
        [res.results[i]["c"] for i in range(N_CORES)], axis=0
    ).astype(np.float32)


# revision 24
# speedup vs baseline: 1.4744x; 1.0148x over previous
"""AttentionContext kernel for Trainium2, data-parallel over batch on 8 cores.

Reference computation (B=64, T=2048, D=512 everywhere):
    phi_s = s @ phi_w.T + phi_b                  # [B, D]
    psi_h = einsum('bth,ah->bta', h, psi_w) + psi_b
    e     = einsum('ba,bta->bt', phi_s, psi_h)   # [B, T]
    alpha = softmax(e, axis=-1)
    c     = alpha * h.sum(-1)                    # [B, T]

Algebraic restructuring:
    e[b,t] = (phi_s[b] @ psi_w) . h[b,t] + const(b); softmax drops const(b).
    w = s @ (phi_w.T @ psi_w) + phi_b @ psi_w    # [B, D], tiny, on PE

Streaming design (per core: 8 batches, 128 tiles of [128, 512]):
  * All DRAM traffic rides one SWDGE (Pool) queue: small fp16-cast weight
    loads first (HWDGE weight loads starve behind a saturated SWDGE h
    stream at the shared SDMA engines), then the h stream. h is cast
    fp32->fp16 inline in the DMA; the t-mapping t = p*16 + j makes each
    partition's slice of a batch one contiguous 32KB DRAM read. Softmax is
    permutation-invariant in t and the output store needs no transpose.
  * The free-dim reductions (e = sum_d h*w and hsum = sum_d h) ride the
    TensorEngine: with a stationary fp16 identity, 16 accumulating
    matmuls per batch fold [128, 16, 512] -> PSUM [128, 16, 32] (each
    rhs chunk is an identity-copy accumulated into the same PSUM bank,
    fp32). One segmented DVE tensor_reduce finishes 32 -> 1 straight out
    of PSUM. The DVE only computes the h*w product (fp16 tensor_tensor at
    2 elem/cycle against a stride-0-broadcast w), in halves interleaved
    with the PE folds to keep the PE clock-gate (HAM) warm.
  * stage 0 runs in fp16 on a HAM-warmed PE so w is broadcast (K=1 matmul
    per batch + PSUM evacuation) before the first batch finishes loading.
    The last batch loads in quarter tiles and folds per quarter to keep
    the post-stream tail short.
  * softmax: per-batch exp(e - colmax_p) on ScalarE with the row max as
    activation bias (negate=True reduce), then one batched cross-partition
    combine: bmax via PE transpose + reduce, correction exp(colmax - bmax)
    folded into the final normalization multiply, single 64KB store.
"""

import numpy as np

import concourse.bass as bass
import concourse.bacc as bacc
import concourse.tile as tile
from concourse import mybir
from concourse import bass_utils
from concourse.masks import make_identity

FP = mybir.dt.float32
F16 = mybir.dt.float16
ALU = mybir.AluOpType
AF = mybir.ActivationFunctionType

N_CORES = 8
B_LOC = 8          # batches per core
T = 2048
D = 512
P = 128
KC = D // P        # 4 contraction chunks of 128
NJ = T // P        # 16 t-tiles per batch (t = p*16 + j)
QW = 32            # PE fold output width: 512 -> QW via D//QW matmuls


def _rep_ap(ap2, n):
    """[P, W] AP -> [P, n, W] view with stride-0 middle dim."""
    return bass.AP(
        tensor=ap2.tensor, offset=ap2.offset, ap=[ap2.ap[0], [0, n], ap2.ap[1]]
    )


def _emit(nc, tc):
    s = nc.dram_tensor("s", [B_LOC, D], FP, kind="ExternalInput").ap()
    h = nc.dram_tensor("h", [B_LOC, T, D], FP, kind="ExternalInput").ap()
    phi_w = nc.dram_tensor("phi_w", [D, D], FP, kind="ExternalInput").ap()
    phi_b = nc.dram_tensor("phi_b", [D], FP, kind="ExternalInput").ap()
    psi_w = nc.dram_tensor("psi_w", [D, D], FP, kind="ExternalInput").ap()
    c_out = nc.dram_tensor("c", [B_LOC, T], FP, kind="ExternalOutput").ap()

    with tc.tile_pool(name="consts", bufs=1) as consts:
        # ------- stage 0 inputs: fp16 cast loads, FIRST on the Pool queue ----
        s_sb = consts.tile([B_LOC, D], FP)
        nc.gpsimd.dma_start(out=s_sb, in_=s)
        phi16_sb = consts.tile([P, KC, D], F16)  # [a % 128, a // 128, k]
        nc.gpsimd.dma_start(
            out=phi16_sb, in_=phi_w.rearrange("(ac p) k -> p ac k", p=P)
        )
        psi16_sb = consts.tile([P, KC, D], F16)  # [a % 128, a // 128, m]
        nc.gpsimd.dma_start(
            out=psi16_sb, in_=psi_w.rearrange("(ac p) k -> p ac k", p=P)
        )
        phi_b16 = consts.tile([P, KC], F16)      # [a % 128, a // 128]
        nc.gpsimd.dma_start(
            out=phi_b16, in_=phi_b.rearrange("(ac p) -> p ac", p=P)
        )

        ident = consts.tile([P, P], FP)
        make_identity(nc, ident)
        ident16 = consts.tile([P, P], F16)
        nc.vector.tensor_copy(out=ident16, in_=ident)
        ones_1x128 = consts.tile([1, P], FP)
        nc.vector.memset(ones_1x128, 1.0)
        neg_1x128 = consts.tile([1, P], FP)
        nc.vector.memset(neg_1x128, -1.0)
        ones_128x1 = consts.tile([P, 1], FP)
        nc.vector.memset(ones_128x1, 1.0)
        ones16_1x128 = consts.tile([1, P], F16)
        nc.vector.memset(ones16_1x128, 1.0)

        # Warm the ACT exp table set early so the ~2.7us load overlaps.
        tiny = consts.tile([1, 1], FP)
        nc.vector.memset(tiny, 0.0)
        nc.scalar.activation(out=tiny, in_=tiny, func=AF.Exp)

        mc_sb = consts.tile([P, KC, D], F16)     # M_c[k, m], k = kc*128 + p
        v_sb = consts.tile([1, D], F16)          # v[m] = phi_b @ psi_w
        sT_sb = consts.tile([P, KC, B_LOC], F16)  # s.T[k, b]
        w_sb16 = consts.tile([B_LOC, D], F16)    # w[b, m] fp16
        w_rows16 = consts.tile([1, B_LOC, D], F16)  # each w row at partition 0
        w_bc16 = consts.tile([P, B_LOC, D], F16)  # w[b] broadcast down parts

        e_all = consts.tile([P, P], FP)          # e[p, b*16 + j], t = p*16+j
        hs_all = consts.tile([P, P], FP)         # hsum, same layout
        exp_all = consts.tile([P, P], FP)        # exp(e - colmax_p)
        ncm_all = consts.tile([P, B_LOC], FP)    # -colmax[p, b]
        pscol_all = consts.tile([P, B_LOC], FP)  # sum_j exp_all per (p, b)
        cmarg = consts.tile([P, B_LOC], FP)
        cmexp = consts.tile([P, B_LOC], FP)
        pscw = consts.tile([P, B_LOC], FP)
        nb_sb = consts.tile([P, B_LOC], FP)
        bmax_sb = consts.tile([B_LOC, 1], FP)
        bt_sb = consts.tile([1, B_LOC], FP)
        rcp_sb = consts.tile([B_LOC, 1], FP)
        rt_sb = consts.tile([1, B_LOC], FP)
        rb_sb = consts.tile([P, B_LOC], FP)
        wf_sb = consts.tile([P, B_LOC], FP)
        cbuf = consts.tile([P, P], FP)

        # HAM warm-up: ~3us of back-to-back PE activity (identity transposes,
        # results discarded) flips the PE clock gate to 8/8 before stage 0.
        with tc.tile_pool(name="psumw", bufs=2, space="PSUM") as psumw:
            for _ in range(10):
                warm_ps = psumw.tile([P, P], FP, tag="warm_ps")
                nc.tensor.transpose(warm_ps, in_=ident, identity=ident)

        with tc.tile_pool(name="psum0", bufs=2, space="PSUM") as psum0:
            for kc in range(KC):
                st_ps = psum0.tile([P, B_LOC], FP, tag="st_ps")
                nc.tensor.transpose(
                    st_ps,
                    in_=s_sb[:, kc * P : (kc + 1) * P],
                    identity=ident[:B_LOC, :B_LOC],
                )
                nc.vector.tensor_copy(out=sT_sb[:, kc, :], in_=st_ps)

            # M_c[k, m] = sum_a phi_w[a, k] * psi_w[a, m]   (fp16 operands)
            for kc in range(KC):
                mc_ps = psum0.tile([P, D], FP, tag="mc_ps")
                for ac in range(KC):
                    nc.tensor.matmul(
                        mc_ps,
                        lhsT=phi16_sb[:, ac, kc * P : (kc + 1) * P],
                        rhs=psi16_sb[:, ac, :],
                        start=(ac == 0),
                        stop=(ac == KC - 1),
                    )
                nc.vector.tensor_copy(out=mc_sb[:, kc, :], in_=mc_ps)

            # v[m] = sum_a phi_b[a] * psi_w[a, m]
            v_ps = psum0.tile([1, D], FP, tag="v_ps")
            for ac in range(KC):
                nc.tensor.matmul(
                    v_ps,
                    lhsT=phi_b16[:, ac : ac + 1],
                    rhs=psi16_sb[:, ac, :],
                    start=(ac == 0),
                    stop=(ac == KC - 1),
                )
            nc.vector.tensor_copy(out=v_sb, in_=v_ps)

            # w[b, m] = sum_k sT[k, b] * M_c[k, m] + 1 * v[m]
            w_ps = psum0.tile([B_LOC, D], FP, tag="w_ps")
            for kc in range(KC):
                nc.tensor.matmul(
                    w_ps,
                    lhsT=sT_sb[:, kc, :],
                    rhs=mc_sb[:, kc, :],
                    start=(kc == 0),
                    stop=False,
                )
            nc.tensor.matmul(
                w_ps, lhsT=ones16_1x128[:, :B_LOC], rhs=v_sb,
                start=False, stop=True,
            )
            nc.vector.tensor_copy(out=w_sb16, in_=w_ps)  # cast fp32 -> fp16

        # relocate each w row to partition 0 (tiny SBUF->SBUF DMA on the
        # sync queue; PE matmul rhs must start at partition 0/32/64), then
        # broadcast it down the 128 partitions with a K=1 fp16 matmul.
        for b in range(B_LOC):
            nc.sync.dma_start(out=w_rows16[:, b, :], in_=w_sb16[b : b + 1, :])
        with tc.tile_pool(name="psum1", bufs=2, space="PSUM") as psum1:
            for b in range(B_LOC):
                bc_ps = psum1.tile([P, D], FP, tag="bc_ps")
                nc.tensor.matmul(
                    bc_ps, lhsT=ones16_1x128, rhs=w_rows16[:, b, :]
                )
                # evacuate on DVE: ScalarE's strict FIFO would order these
                # behind early hsum activations, starving the e-products
                nc.vector.tensor_copy(out=w_bc16[:, b, :], in_=bc_ps)

        # ---------------- stage 1: stream h ----------------
        with (
            tc.tile_pool(name="hpool", bufs=4) as hpool,
            tc.tile_pool(name="qpool", bufs=4) as qpool,
            tc.tile_pool(name="ppool", bufs=2) as ppool,
            tc.tile_pool(name="psum3", bufs=2, space="PSUM") as psum3,
        ):
            QN = NJ // 4

            def do_block(src3, b, j0, n):
                """hsum + e for tiles [j0, j0+n) of batch b from src3.
                Folds [P, n, 512] -> PSUM [P, n, 64] via 8 accumulating
                identity matmuls, finishes 64 -> 1 with one segmented
                tensor_reduce straight out of PSUM."""
                cols = slice(b * NJ + j0, b * NJ + j0 + n)
                hs_ps = psum3.tile([P, n, 64], FP, tag=f"hs_ps{n}")
                for ci in range(8):
                    nc.tensor.matmul(
                        hs_ps, lhsT=ident16,
                        rhs=src3[:, :, ci * 64 : (ci + 1) * 64],
                        start=(ci == 0), stop=(ci == 7),
                    )
                nc.vector.tensor_reduce(
                    out=hs_all[:, cols], in_=hs_ps,
                    axis=mybir.AxisListType.X, op=ALU.add,
                )
                prod = ppool.tile([P, n, D], F16, tag=f"prod{n}")
                nc.vector.tensor_tensor(
                    out=prod, in0=src3, in1=_rep_ap(w_bc16[:, b, :], n),
                    op=ALU.mult,
                )
                e_ps = psum3.tile([P, n, 64], FP, tag=f"e_ps{n}")
                for ci in range(8):
                    nc.tensor.matmul(
                        e_ps, lhsT=ident16,
                        rhs=prod[:, :, ci * 64 : (ci + 1) * 64],
                        start=(ci == 0), stop=(ci == 7),
                    )
                nc.vector.tensor_reduce(
                    out=e_all[:, cols], in_=e_ps,
                    axis=mybir.AxisListType.X, op=ALU.add,
                )

            for b in range(B_LOC):
                c0 = b * NJ
                last = b == B_LOC - 1
                if last:
                    # quarter tiles: precise deps so the tail work starts
                    # as each quarter lands, not after the full batch
                    h3 = h[b].rearrange("(p j) d -> p j d", p=P)
                    for v in range(4):
                        hq = qpool.tile([P, QN, D], F16, tag=f"hq{v}")
                        nc.gpsimd.dma_start(
                            out=hq, in_=h3[:, v * QN : (v + 1) * QN, :]
                        )
                        do_block(hq, b, v * QN, QN)
                else:
                    ht = hpool.tile([P, NJ, D], F16, tag="ht")
                    nc.gpsimd.dma_start(
                        out=ht, in_=h[b].rearrange("(p j) d -> p j d", p=P)
                    )
                    # halves: the [P, 8, 64] fold tiles are one PSUM bank
                    # each, and half-granular prod/fold interleaving keeps
                    # the PE HAM-warm between batches
                    do_block(ht[:, 0 : NJ // 2, :], b, 0, NJ // 2)
                    do_block(ht[:, NJ // 2 :, :], b, NJ // 2, NJ // 2)

                # ---- per-batch: row max and exp(e - colmax_p) ----
                nc.vector.tensor_reduce(
                    out=ncm_all[:, b : b + 1], in_=e_all[:, c0 : c0 + NJ],
                    axis=mybir.AxisListType.X, op=ALU.max, negate=True,
                )
                nc.scalar.activation(
                    out=exp_all[:, c0 : c0 + NJ], in_=e_all[:, c0 : c0 + NJ],
                    func=AF.Exp, bias=ncm_all[:, b : b + 1], scale=1.0,
                    accum_out=pscol_all[:, b : b + 1],
                )

        # ---------------- stage 2: batched softmax combine ----------------
        with tc.tile_pool(name="psum2", bufs=1, space="PSUM") as psum2:
            # bmax[b] = max_p colmax[p, b]; ncm = -colmax
            cmT_ps = psum2.tile([B_LOC, P], FP, tag="cmT_ps")
            nc.tensor.transpose(cmT_ps, in_=ncm_all, identity=ident)
            nc.vector.tensor_reduce(
                out=bmax_sb, in_=cmT_ps, axis=mybir.AxisListType.X,
                op=ALU.min, negate=True,
            )
            bt_ps = psum2.tile([1, B_LOC], FP, tag="bt_ps")
            nc.tensor.transpose(
                bt_ps, in_=bmax_sb, identity=ident[:B_LOC, :B_LOC]
            )
            nc.vector.tensor_copy(out=bt_sb, in_=bt_ps)
            # -bmax broadcast down partitions
            nb_ps = psum2.tile([P, B_LOC], FP, tag="nb_ps")
            nc.tensor.matmul(nb_ps, lhsT=neg_1x128, rhs=bt_sb)
            nc.vector.tensor_copy(out=nb_sb, in_=nb_ps)
            # cmarg = colmax - bmax = nb - ncm
            nc.vector.tensor_tensor(
                out=cmarg, in0=nb_sb, in1=ncm_all, op=ALU.subtract
            )
            nc.scalar.activation(out=cmexp, in_=cmarg, func=AF.Exp)
            nc.vector.tensor_tensor(
                out=pscw, in0=pscol_all, in1=cmexp, op=ALU.mult
            )
            # Z[b] = sum_p pscw[p, b]
            z_ps = psum2.tile([B_LOC, 1], FP, tag="z_ps")
            nc.tensor.matmul(z_ps, lhsT=pscw, rhs=ones_128x1)
            nc.vector.reciprocal(out=rcp_sb, in_=z_ps)
            rt_ps = psum2.tile([1, B_LOC], FP, tag="rt_ps")
            nc.tensor.transpose(
                rt_ps, in_=rcp_sb, identity=ident[:B_LOC, :B_LOC]
            )
            nc.vector.tensor_copy(out=rt_sb, in_=rt_ps)
            rb_ps = psum2.tile([P, B_LOC], FP, tag="rb_ps")
            nc.tensor.matmul(rb_ps, lhsT=ones_1x128, rhs=rt_sb)
            nc.vector.tensor_copy(out=rb_sb, in_=rb_ps)
            nc.vector.tensor_tensor(out=wf_sb, in0=cmexp, in1=rb_sb, op=ALU.mult)

            # c = exp_all * hs_all * wf[p, b]  (wf broadcast over j)
            nc.vector.tensor_tensor(
                out=cbuf, in0=exp_all, in1=hs_all, op=ALU.mult
            )
            wf_rep = bass.AP(
                tensor=wf_sb.tensor, offset=wf_sb.offset,
                ap=[wf_sb.ap[0], wf_sb.ap[1], [0, NJ]],
            )
            cbuf3 = bass.AP(
                tensor=cbuf.tensor, offset=cbuf.offset,
                ap=[cbuf.ap[0], [NJ, B_LOC], [1, NJ]],
            )
            nc.vector.tensor_tensor(
                out=cbuf3, in0=cbuf3, in1=wf_rep, op=ALU.mult
            )
            # c[b, p*16 + j] = cbuf[p, b*16 + j]. The DRAM pattern is 1024
            # 64B segments (RMW-penalized), so split across both HWDGE
            # rings to halve the serial store time.
            half_c = B_LOC // 2 * NJ
            nc.sync.dma_start(
                out=c_out[: B_LOC // 2].rearrange("b (p j) -> p b j", p=P),
                in_=cbuf[:, :half_c],
            )
            nc.scalar.dma_start(
                out=c_out[B_LOC // 2 :].rearrange("b (p j) -> p b j", p=P),
                in_=cbuf[:, half_c:],
            )


_CACHE = {}


def _build():
    if "nc" not in _CACHE:
        nc = bacc.Bacc(
            "TRN2", target_bir_lowering=False, debug=False, num_devices=N_CORES
        )
        with tile.TileContext(nc) as tc:
            _emit(nc, tc)
        nc.compile()
        _CACHE["nc"] = nc
    return _CACHE["nc"]


def kernel(s, h, phi_w, phi_b, psi_w, psi_b=None, **_unused):
    s = np.ascontiguousarray(np.asarray(s, dtype=np.float32))
    h = np.ascontiguousarray(np.asarray(h, dtype=np.float32))
    phi_w = np.ascontiguousarray(np.asarray(phi_w, dtype=np.float32))
    phi_b = np.ascontiguousarray(np.asarray(phi_b, dtype=np.float32))
    psi_w = np.ascontiguousarray(np.asarray(psi_w, dtype=np.float32))

    nc = _build()
    in_maps = [
        {
            "s": s[i * B_LOC : (i + 1) * B_LOC],
            "h": h[i * B_LOC : (i + 1) * B_LOC],
            "phi_w": phi_w,
            "phi_b": phi_b,
            "psi_w": psi_w,
        }
        for i in range(N_CORES)
    ]
    res = bass_utils.run_bass_kernel_spmd(nc, in_maps, core_ids=list(range(N_CORES)))
    return np.concatenate(
        [res.results[i]["c"] for i in range(N_CORES)], axis=0
    ).astype(np.float32)


# revision 28
# speedup vs baseline: 1.6011x; 1.0859x over previous
"""AttentionContext kernel for Trainium2, data-parallel over batch on 8 cores.

Reference computation (B=64, T=2048, D=512 everywhere):
    phi_s = s @ phi_w.T + phi_b                  # [B, D]
    psi_h = einsum('bth,ah->bta', h, psi_w) + psi_b
    e     = einsum('ba,bta->bt', phi_s, psi_h)   # [B, T]
    alpha = softmax(e, axis=-1)
    c     = alpha * h.sum(-1)                    # [B, T]

Algebraic restructuring:
    e[b,t] = (phi_s[b] @ psi_w) . h[b,t] + const(b); softmax drops const(b).
    w = s @ (phi_w.T @ psi_w) + phi_b @ psi_w    # [B, D], tiny, on PE

Streaming design (per core: 8 batches, 128 tiles of [128, 512]):
  * All DRAM traffic rides one SWDGE (Pool) queue: small fp16-cast weight
    loads first (HWDGE weight loads starve behind a saturated SWDGE h
    stream at the shared SDMA engines), then the h stream. h is cast
    fp32->fp16 inline in the DMA; the t-mapping t = p*16 + j makes each
    partition's slice of a batch one contiguous 32KB DRAM read. Softmax is
    permutation-invariant in t and the output store needs no transpose.
  * The free-dim reductions (e = sum_d h*w and hsum = sum_d h) ride the
    TensorEngine: with a stationary fp16 identity, 16 accumulating
    matmuls per batch fold [128, 16, 512] -> PSUM [128, 16, 32] (each
    rhs chunk is an identity-copy accumulated into the same PSUM bank,
    fp32). One segmented DVE tensor_reduce finishes 32 -> 1 straight out
    of PSUM. The DVE only computes the h*w product (fp16 tensor_tensor at
    2 elem/cycle against a stride-0-broadcast w), in halves interleaved
    with the PE folds to keep the PE clock-gate (HAM) warm.
  * stage 0 runs in fp16 on a HAM-warmed PE so w is broadcast (K=1 matmul
    per batch + PSUM evacuation) before the first batch finishes loading.
    The last batch loads in quarter tiles and folds per quarter to keep
    the post-stream tail short.
  * softmax: per-batch exp(e - colmax_p) on ScalarE with the row max as
    activation bias (negate=True reduce), then one batched cross-partition
    combine: bmax via PE transpose + reduce, correction exp(colmax - bmax)
    folded into the final normalization multiply, single 64KB store.
"""

import numpy as np

import concourse.bass as bass
import concourse.bacc as bacc
import concourse.tile as tile
from concourse import mybir
from concourse import bass_utils
from concourse.masks import make_identity

FP = mybir.dt.float32
F16 = mybir.dt.float16
ALU = mybir.AluOpType
AF = mybir.ActivationFunctionType

N_CORES = 8
B_LOC = 8          # batches per core
T = 2048
D = 512
P = 128
KC = D // P        # 4 contraction chunks of 128
NJ = T // P        # 16 t-tiles per batch (t = p*16 + j)
QW = 32            # PE fold output width: 512 -> QW via D//QW matmuls


def _rep_ap(ap2, n):
    """[P, W] AP -> [P, n, W] view with stride-0 middle dim."""
    return bass.AP(
        tensor=ap2.tensor, offset=ap2.offset, ap=[ap2.ap[0], [0, n], ap2.ap[1]]
    )


def _emit(nc, tc):
    s = nc.dram_tensor("s", [B_LOC, D], FP, kind="ExternalInput").ap()
    h = nc.dram_tensor("h", [B_LOC, T, D], FP, kind="ExternalInput").ap()
    phi_w = nc.dram_tensor("phi_w", [D, D], FP, kind="ExternalInput").ap()
    phi_b = nc.dram_tensor("phi_b", [D], FP, kind="ExternalInput").ap()
    psi_w = nc.dram_tensor("psi_w", [D, D], FP, kind="ExternalInput").ap()
    c_out = nc.dram_tensor("c", [B_LOC, T], FP, kind="ExternalOutput").ap()

    with tc.tile_pool(name="consts", bufs=1) as consts:
        # identity first: make_identity's GpSimd ops must precede the DMA
        # dispatches in the Pool queue or the PE warm-up starts ~5us late
        ident = consts.tile([P, P], FP)
        make_identity(nc, ident)
        ident16 = consts.tile([P, P], F16)
        nc.vector.tensor_copy(out=ident16, in_=ident)

        # ------- stage 0 inputs: fp16 cast loads, FIRST on the Pool queue ----
        s_sb = consts.tile([B_LOC, D], FP)
        nc.gpsimd.dma_start(out=s_sb, in_=s)
        phi16_sb = consts.tile([P, KC, D], F16)  # [a % 128, a // 128, k]
        nc.gpsimd.dma_start(
            out=phi16_sb, in_=phi_w.rearrange("(ac p) k -> p ac k", p=P)
        )
        psi16_sb = consts.tile([P, KC, D], F16)  # [a % 128, a // 128, m]
        nc.gpsimd.dma_start(
            out=psi16_sb, in_=psi_w.rearrange("(ac p) k -> p ac k", p=P)
        )
        phi_b16 = consts.tile([P, KC], F16)      # [a % 128, a // 128]
        nc.gpsimd.dma_start(
            out=phi_b16, in_=phi_b.rearrange("(ac p) -> p ac", p=P)
        )
        ones_1x128 = consts.tile([1, P], FP)
        nc.vector.memset(ones_1x128, 1.0)
        neg_1x128 = consts.tile([1, P], FP)
        nc.vector.memset(neg_1x128, -1.0)
        ones_128x1 = consts.tile([P, 1], FP)
        nc.vector.memset(ones_128x1, 1.0)
        ones16_1x128 = consts.tile([1, P], F16)
        nc.vector.memset(ones16_1x128, 1.0)

        # Warm the ACT exp table set early so the ~2.7us load overlaps.
        tiny = consts.tile([1, 1], FP)
        nc.vector.memset(tiny, 0.0)
        nc.scalar.activation(out=tiny, in_=tiny, func=AF.Exp)

        mc_sb = consts.tile([P, KC, D], F16)     # M_c[k, m], k = kc*128 + p
        v_sb = consts.tile([1, D], F16)          # v[m] = phi_b @ psi_w
        sT_sb = consts.tile([P, KC, B_LOC], F16)  # s.T[k, b]
        w_sb16 = consts.tile([B_LOC, D], F16)    # w[b, m] fp16
        w_rows16 = consts.tile([1, B_LOC, D], F16)  # each w row at partition 0
        w_bc16 = consts.tile([P, B_LOC, D], F16)  # w[b] broadcast down parts

        e_all = consts.tile([P, P], FP)          # e[p, b*16 + j], t = p*16+j
        hs_all = consts.tile([P, P], FP)         # hsum, same layout
        exp_all = consts.tile([P, P], FP)        # exp(e - colmax_p)
        ncm_all = consts.tile([P, B_LOC], FP)    # -colmax[p, b]
        pscol_all = consts.tile([P, B_LOC], FP)  # sum_j exp_all per (p, b)
        cmarg = consts.tile([P, B_LOC], FP)
        cmexp = consts.tile([P, B_LOC], FP)
        pscw = consts.tile([P, B_LOC], FP)
        nb_sb = consts.tile([P, B_LOC], FP)
        bmax_sb = consts.tile([B_LOC, 1], FP)
        bt_sb = consts.tile([1, B_LOC], FP)
        rcp_sb = consts.tile([B_LOC, 1], FP)
        rt_sb = consts.tile([1, B_LOC], FP)
        rb_sb = consts.tile([P, B_LOC], FP)
        wf_sb = consts.tile([P, B_LOC], FP)
        cbuf = consts.tile([P, P], FP)

        # HAM warm-up: ~3us of back-to-back REAL matmuls (transpose-mode
        # does not count as PE-busy for HAM) flips the clock gate to 8/8
        # before the stage-0 chain; results are discarded.
        with tc.tile_pool(name="psumw", bufs=2, space="PSUM") as psumw:
            for _ in range(12):
                warm_ps = psumw.tile([P, P], FP, tag="warm_ps")
                nc.tensor.matmul(warm_ps, lhsT=ident16, rhs=ident16)

        with tc.tile_pool(name="psum0", bufs=2, space="PSUM") as psum0:
            for kc in range(KC):
                st_ps = psum0.tile([P, B_LOC], FP, tag="st_ps")
                nc.tensor.transpose(
                    st_ps,
                    in_=s_sb[:, kc * P : (kc + 1) * P],
                    identity=ident[:B_LOC, :B_LOC],
                )
                nc.vector.tensor_copy(out=sT_sb[:, kc, :], in_=st_ps)

            # M_c[k, m] = sum_a phi_w[a, k] * psi_w[a, m]   (fp16 operands)
            for kc in range(KC):
                mc_ps = psum0.tile([P, D], FP, tag="mc_ps")
                for ac in range(KC):
                    nc.tensor.matmul(
                        mc_ps,
                        lhsT=phi16_sb[:, ac, kc * P : (kc + 1) * P],
                        rhs=psi16_sb[:, ac, :],
                        start=(ac == 0),
                        stop=(ac == KC - 1),
                    )
                nc.vector.tensor_copy(out=mc_sb[:, kc, :], in_=mc_ps)

            # v[m] = sum_a phi_b[a] * psi_w[a, m]
            v_ps = psum0.tile([1, D], FP, tag="v_ps")
            for ac in range(KC):
                nc.tensor.matmul(
                    v_ps,
                    lhsT=phi_b16[:, ac : ac + 1],
                    rhs=psi16_sb[:, ac, :],
                    start=(ac == 0),
                    stop=(ac == KC - 1),
                )
            nc.vector.tensor_copy(out=v_sb, in_=v_ps)

            # w[b, m] = sum_k sT[k, b] * M_c[k, m] + 1 * v[m]
            w_ps = psum0.tile([B_LOC, D], FP, tag="w_ps")
            for kc in range(KC):
                nc.tensor.matmul(
                    w_ps,
                    lhsT=sT_sb[:, kc, :],
                    rhs=mc_sb[:, kc, :],
                    start=(kc == 0),
                    stop=False,
                )
            nc.tensor.matmul(
                w_ps, lhsT=ones16_1x128[:, :B_LOC], rhs=v_sb,
                start=False, stop=True,
            )
            nc.vector.tensor_copy(out=w_sb16, in_=w_ps)  # cast fp32 -> fp16

        # relocate each w row to partition 0 (tiny SBUF->SBUF DMA on the
        # sync queue; PE matmul rhs must start at partition 0/32/64), then
        # broadcast it down the 128 partitions with a K=1 fp16 matmul.
        for b in range(B_LOC):
            nc.sync.dma_start(out=w_rows16[:, b, :], in_=w_sb16[b : b + 1, :])
        # bufs=8 (one bank each): all 8 broadcast matmuls run back-to-back
        # instead of ping-ponging with their DVE evacuations through 2 banks
        with tc.tile_pool(name="psum1", bufs=8, space="PSUM") as psum1:
            for b in range(B_LOC):
                bc_ps = psum1.tile([P, D], FP, tag="bc_ps")
                nc.tensor.matmul(
                    bc_ps, lhsT=ones16_1x128, rhs=w_rows16[:, b, :]
                )
                # evacuate on DVE: ScalarE's strict FIFO would order these
                # behind early hsum activations, starving the e-products
                nc.vector.tensor_copy(out=w_bc16[:, b, :], in_=bc_ps)

        # ---------------- stage 1: stream h ----------------
        with (
            tc.tile_pool(name="hpool", bufs=4) as hpool,
            tc.tile_pool(name="qpool", bufs=4) as qpool,
            tc.tile_pool(name="ppool", bufs=2) as ppool,
            tc.tile_pool(name="psum3", bufs=2, space="PSUM") as psum3,
        ):
            QN = NJ // 4

            def do_block(src3, b, j0, n):
                """hsum + e for tiles [j0, j0+n) of batch b from src3.
                Folds [P, n, 512] -> PSUM [P, n, 64] via 8 accumulating
                identity matmuls, finishes 64 -> 1 with one segmented
                tensor_reduce straight out of PSUM."""
                cols = slice(b * NJ + j0, b * NJ + j0 + n)
                hs_ps = psum3.tile([P, n, 64], FP, tag=f"hs_ps{n}")
                for ci in range(8):
                    nc.tensor.matmul(
                        hs_ps, lhsT=ident16,
                        rhs=src3[:, :, ci * 64 : (ci + 1) * 64],
                        start=(ci == 0), stop=(ci == 7),
                    )
                nc.vector.tensor_reduce(
                    out=hs_all[:, cols], in_=hs_ps,
                    axis=mybir.AxisListType.X, op=ALU.add,
                )
                prod = ppool.tile([P, n, D], F16, tag=f"prod{n}")
                nc.vector.tensor_tensor(
                    out=prod, in0=src3, in1=_rep_ap(w_bc16[:, b, :], n),
                    op=ALU.mult,
                )
                e_ps = psum3.tile([P, n, 64], FP, tag=f"e_ps{n}")
                for ci in range(8):
                    nc.tensor.matmul(
                        e_ps, lhsT=ident16,
                        rhs=prod[:, :, ci * 64 : (ci + 1) * 64],
                        start=(ci == 0), stop=(ci == 7),
                    )
                nc.vector.tensor_reduce(
                    out=e_all[:, cols], in_=e_ps,
                    axis=mybir.AxisListType.X, op=ALU.add,
                )

            for b in range(B_LOC):
                c0 = b * NJ
                last = b == B_LOC - 1
                if last:
                    # quarter tiles: precise deps so the tail work starts
                    # as each quarter lands, not after the full batch
                    h3 = h[b].rearrange("(p j) d -> p j d", p=P)
                    for v in range(4):
                        hq = qpool.tile([P, QN, D], F16, tag=f"hq{v}")
                        nc.gpsimd.dma_start(
                            out=hq, in_=h3[:, v * QN : (v + 1) * QN, :]
                        )
                        do_block(hq, b, v * QN, QN)
                else:
                    ht = hpool.tile([P, NJ, D], F16, tag="ht")
                    nc.gpsimd.dma_start(
                        out=ht, in_=h[b].rearrange("(p j) d -> p j d", p=P)
                    )
                    # halves: the [P, 8, 64] fold tiles are one PSUM bank
                    # each, and half-granular prod/fold interleaving keeps
                    # the PE HAM-warm between batches
                    do_block(ht[:, 0 : NJ // 2, :], b, 0, NJ // 2)
                    do_block(ht[:, NJ // 2 :, :], b, NJ // 2, NJ // 2)

                # ---- per-batch: row max and exp(e - colmax_p) ----
                nc.vector.tensor_reduce(
                    out=ncm_all[:, b : b + 1], in_=e_all[:, c0 : c0 + NJ],
                    axis=mybir.AxisListType.X, op=ALU.max, negate=True,
                )
                nc.scalar.activation(
                    out=exp_all[:, c0 : c0 + NJ], in_=e_all[:, c0 : c0 + NJ],
                    func=AF.Exp, bias=ncm_all[:, b : b + 1], scale=1.0,
                    accum_out=pscol_all[:, b : b + 1],
                )

        # ---------------- stage 2: batched softmax combine ----------------
        with tc.tile_pool(name="psum2", bufs=1, space="PSUM") as psum2:
            # bmax[b] = max_p colmax[p, b]; ncm = -colmax
            cmT_ps = psum2.tile([B_LOC, P], FP, tag="cmT_ps")
            nc.tensor.transpose(cmT_ps, in_=ncm_all, identity=ident)
            nc.vector.tensor_reduce(
                out=bmax_sb, in_=cmT_ps, axis=mybir.AxisListType.X,
                op=ALU.min, negate=True,
            )
            bt_ps = psum2.tile([1, B_LOC], FP, tag="bt_ps")
            nc.tensor.transpose(
                bt_ps, in_=bmax_sb, identity=ident[:B_LOC, :B_LOC]
            )
            nc.vector.tensor_copy(out=bt_sb, in_=bt_ps)
            # -bmax broadcast down partitions
            nb_ps = psum2.tile([P, B_LOC], FP, tag="nb_ps")
            nc.tensor.matmul(nb_ps, lhsT=neg_1x128, rhs=bt_sb)
            nc.vector.tensor_copy(out=nb_sb, in_=nb_ps)
            # cmarg = colmax - bmax = nb - ncm
            nc.vector.tensor_tensor(
                out=cmarg, in0=nb_sb, in1=ncm_all, op=ALU.subtract
            )
            nc.scalar.activation(out=cmexp, in_=cmarg, func=AF.Exp)
            nc.vector.tensor_tensor(
                out=pscw, in0=pscol_all, in1=cmexp, op=ALU.mult
            )
            # Z[b] = sum_p pscw[p, b]
            z_ps = psum2.tile([B_LOC, 1], FP, tag="z_ps")
            nc.tensor.matmul(z_ps, lhsT=pscw, rhs=ones_128x1)
            nc.vector.reciprocal(out=rcp_sb, in_=z_ps)
            rt_ps = psum2.tile([1, B_LOC], FP, tag="rt_ps")
            nc.tensor.transpose(
                rt_ps, in_=rcp_sb, identity=ident[:B_LOC, :B_LOC]
            )
            nc.vector.tensor_copy(out=rt_sb, in_=rt_ps)
            rb_ps = psum2.tile([P, B_LOC], FP, tag="rb_ps")
            nc.tensor.matmul(rb_ps, lhsT=ones_1x128, rhs=rt_sb)
            nc.vector.tensor_copy(out=rb_sb, in_=rb_ps)
            nc.vector.tensor_tensor(out=wf_sb, in0=cmexp, in1=rb_sb, op=ALU.mult)

            # c = exp_all * hs_all * wf[p, b]  (wf broadcast over j)
            nc.vector.tensor_tensor(
                out=cbuf, in0=exp_all, in1=hs_all, op=ALU.mult
            )
            wf_rep = bass.AP(
                tensor=wf_sb.tensor, offset=wf_sb.offset,
                ap=[wf_sb.ap[0], wf_sb.ap[1], [0, NJ]],
            )
            cbuf3 = bass.AP(
                tensor=cbuf.tensor, offset=cbuf.offset,
                ap=[cbuf.ap[0], [NJ, B_LOC], [1, NJ]],
            )
            nc.vector.tensor_tensor(
                out=cbuf3, in0=cbuf3, in1=wf_rep, op=ALU.mult
            )
            # c[b, p*16 + j] = cbuf[p, b*16 + j]. The DRAM pattern is 1024
            # 64B segments (RMW-penalized), so split across both HWDGE
            # rings plus the (now idle) SWDGE ring to cut the serial time.
            for lane, (engine, b0r, b1r) in enumerate(
                [(nc.sync, 0, 3), (nc.scalar, 3, 6), (nc.gpsimd, 6, 8)]
            ):
                engine.dma_start(
                    out=c_out[b0r:b1r].rearrange("b (p j) -> p b j", p=P),
                    in_=cbuf[:, b0r * NJ : b1r * NJ],
                )


_CACHE = {}


def _build():
    if "nc" not in _CACHE:
        nc = bacc.Bacc(
            "TRN2", target_bir_lowering=False, debug=False, num_devices=N_CORES
        )
        with tile.TileContext(nc) as tc:
            _emit(nc, tc)
        nc.compile()
        _CACHE["nc"] = nc
    return _CACHE["nc"]


def kernel(s, h, phi_w, phi_b, psi_w, psi_b=None, **_unused):
    s = np.ascontiguousarray(np.asarray(s, dtype=np.float32))
    h = np.ascontiguousarray(np.asarray(h, dtype=np.float32))
    phi_w = np.ascontiguousarray(np.asarray(phi_w, dtype=np.float32))
    phi_b = np.ascontiguousarray(np.asarray(phi_b, dtype=np.float32))
    psi_w = np.ascontiguousarray(np.asarray(psi_w, dtype=np.float32))

    nc = _build()
    in_maps = [
        {
            "s": s[i * B_LOC : (i + 1) * B_LOC],
            "h": h[i * B_LOC : (i + 1) * B_LOC],
            "phi_w": phi_w,
            "phi_b": phi_b,
            "psi_w": psi_w,
        }
        for i in range(N_CORES)
    ]
    res = bass_utils.run_bass_kernel_spmd(nc, in_maps, core_ids=list(range(N_CORES)))
    return np.concatenate(
        [res.results[i]["c"] for i in range(N_CORES)], axis=0
    ).astype(np.float32)
